# revision 1
# baseline (speedup 1.0000x reference)
"""Causal self-attention with RoPE, tensor-parallel over heads on 8 TRN2 NeuronCores.

Model (from the reference):
    q/k/v = x @ W{q,k,v}.T          x: (1, 2048, 2048), 16 heads x 128 head_dim
    rope(q), rope(k)                half-rotation, 32 nonzero freqs
    causal softmax(q k^T / sqrt(128)) @ v
    out = (y / 3) @ Wo.T

Sharding: 2 heads per core. Each core computes its heads' q/k/v projections,
attention, and a partial c_proj (its 256 columns of the hd contraction);
the host sums the 8 partial outputs (the "all-reduce after c_proj").

Per-core kernel layout choices:
  - Everything transposed so the contraction dim is always on partitions:
    host supplies xT (D, T) plus pre-transposed weight slices.
  - Scores computed transposed (S^T[j, i] blocks) so the P @ V matmul needs
    no transposes: OT[d, i] = sum_j V[j, d]^T P^T[j, i] is produced directly
    in the layout c_proj wants.
  - Softmax without max-subtraction (scores are provably tiny: |s| < ~2),
    denominator via DVE accumulation + one all-ones matmul (broadcast sum).
  - RoPE in transposed layout via a 64-partition roll matmul + 3 DVE ops.
  - All matmuls in float32r (full PE rate at moving dim >= 256).
"""

import numpy as np

T = 2048
D = 2048
H = 16
DH = 128
N_CORES = 8
H_LOC = H // N_CORES          # heads per core = 2
HD_LOC = H_LOC * DH           # local head dims = 256
TCH = 512                     # query-chunk width
N_CH = T // TCH               # 4 chunks
KO = D // 128                 # 16 contraction subtiles
XP = 2                        # xT streamed in pieces of 2 k-subtiles
SCALE = (DH ** 0.5) / DH      # 1/sqrt(128)

_CACHE = {}


def build_program():
    """Build (once) the single-core Bass program shared by all 8 cores."""
    if "nc" in _CACHE:
        return _CACHE["nc"]

    from contextlib import ExitStack

    import concourse.bacc as bacc
    import concourse.mybir as mybir
    import concourse.tile as tile

    f32 = mybir.dt.float32
    f32r = mybir.dt.float32r
    bf16 = mybir.dt.bfloat16
    EXP = mybir.ActivationFunctionType.Exp

    nc = bacc.Bacc("TRN2", target_bir_lowering=False)

    xT_d = nc.dram_tensor("xT", (D, T), f32r, kind="ExternalInput")
    wq_d = nc.dram_tensor("wqT", (D, HD_LOC), f32r, kind="ExternalInput")
    wk_d = nc.dram_tensor("wkT", (D, HD_LOC), f32r, kind="ExternalInput")
    wv_d = nc.dram_tensor("wvT", (D, HD_LOC), f32r, kind="ExternalInput")
    wo_d = nc.dram_tensor("woT", (HD_LOC, D), f32r, kind="ExternalInput")
    ct_d = nc.dram_tensor("ctab", (128, T), f32, kind="ExternalInput")
    st_d = nc.dram_tensor("stab", (128, T), f32, kind="ExternalInput")
    roll_d = nc.dram_tensor("roll", (128, 128), f32r, kind="ExternalInput")
    ones_d = nc.dram_tensor("ones", (128, 128), f32r, kind="ExternalInput")
    tri_d = nc.dram_tensor("tri", (128, 128), f32r, kind="ExternalInput")
    out_d = nc.dram_tensor("outp", (T, D), f32, kind="ExternalOutput")

    xT_r = xT_d[:].rearrange("(ko p) t -> p ko t", p=128)
    wq_r = wq_d[:].rearrange("(ko p) m -> p ko m", p=128)
    wk_r = wk_d[:].rearrange("(ko p) m -> p ko m", p=128)
    wv_r = wv_d[:].rearrange("(ko p) m -> p ko m", p=128)
    wo_r = wo_d[:].rearrange("(h p) d -> p h d", p=128)

    with tile.TileContext(nc) as tc, ExitStack() as ctx:
        persist = ctx.enter_context(tc.tile_pool(name="persist", bufs=1))
        qpool = ctx.enter_context(tc.tile_pool(name="qpool", bufs=2))
        ypool = ctx.enter_context(tc.tile_pool(name="ypool", bufs=2))
        xpool = ctx.enter_context(tc.tile_pool(name="xpool", bufs=10))
        ptpool = ctx.enter_context(tc.tile_pool(name="ptpool", bufs=3))
        rtmp = ctx.enter_context(tc.tile_pool(name="rtmp", bufs=1))
        spool = ctx.enter_context(tc.tile_pool(name="spool", bufs=2))
        opool = ctx.enter_context(tc.tile_pool(name="opool", bufs=6))
        psum_p = ctx.enter_context(tc.tile_pool(name="psum_p", bufs=2, space="PSUM"))
        psum_mix = ctx.enter_context(tc.tile_pool(name="psum_mix", bufs=2, space="PSUM"))
        psum_ot = ctx.enter_context(tc.tile_pool(name="psum_ot", bufs=2, space="PSUM"))

        def ps_tile(pool=None):
            return (pool or psum_p).tile([128, TCH], f32, tag="ps", name="ps")

        def mix_tile():
            return psum_mix.tile([128, H_LOC, TCH], f32, tag="mix", name="mix")

        # --- resident tensors ---
        w_q = persist.tile([128, KO, HD_LOC], f32r, tag="w_q")
        w_k = persist.tile([128, KO, HD_LOC], f32r, tag="w_k")
        w_v = persist.tile([128, KO, HD_LOC], f32r, tag="w_v")
        w_o = persist.tile([128, H_LOC, D], f32r, tag="w_o")
        kt = persist.tile([128, H_LOC, T], f32r, tag="kt")
        vt = persist.tile([128, KO, HD_LOC], f32r, tag="vt")
        ctab = persist.tile([128, T], f32, tag="ctab")
        stab = persist.tile([128, T], f32, tag="stab")
        roll = persist.tile([128, 128], f32r, tag="roll")
        ones = persist.tile([128, 128], f32r, tag="ones")
        tri = persist.tile([128, 128], f32r, tag="tri")

        def issue_x(c):
            """Queue the xT piece DMAs for chunk c (weights too on chunk 0)."""
            cs = c * TCH
            pieces = []
            for kp in range(KO // XP):
                ksl = slice(kp * XP, (kp + 1) * XP)
                xc = xpool.tile([128, XP, TCH], f32r, tag="xc", name="xc")
                nc.sync.dma_start(xc[:], xT_r[:, ksl, cs:cs + TCH])
                pieces.append(xc)
                if c == 0:
                    nc.sync.dma_start(w_q[:, ksl, :], wq_r[:, ksl, :])
                    nc.sync.dma_start(w_k[:, ksl, :], wk_r[:, ksl, :])
                    nc.sync.dma_start(w_v[:, ksl, :], wv_r[:, ksl, :])
            if c == 0:
                nc.sync.dma_start(ctab[:], ct_d[:])
                nc.sync.dma_start(stab[:], st_d[:])
                nc.sync.dma_start(roll[:], roll_d[:])
                nc.sync.dma_start(ones[:], ones_d[:])
                nc.sync.dma_start(tri[:], tri_d[:])
            return pieces

        def proj_chunk(c, pieces, only=None, qc=None):
            """q/k/v projections + RoPE for t-chunk c.

            only="q": just the q projection + its rope (enables starting the
            chunk's early attention j-tiles before k/v exist).
            only="kv": the rest. None: everything."""
            cs = c * TCH
            if only != "kv":
                qc = qpool.tile([128, H_LOC, TCH], f32r, tag="qc", name="qc")
            wd = {"q": ((w_q, qc),), "kv": ((w_k, kt),)}.get(only,
                                                            ((w_q, qc), (w_k, kt)))
            for w_sb, dst in wd:
                for h in range(H_LOC):
                    dsl = dst[:, h, :] if dst is qc else dst[:, h, cs:cs + TCH]
                    # k-groups accumulate in the attention ot pool (idle during
                    # projections) so q/k/roll don't serialize through psum_p;
                    # in split mode that pool is live -- fall back to psum_p
                    ps = ps_tile(psum_ot if (dst is kt and only is None) else None)
                    for ko in range(KO):
                        nc.tensor.matmul(
                            ps,
                            lhsT=w_sb[:, ko, h * 128:(h + 1) * 128],
                            rhs=pieces[ko // XP][:, ko % XP, :],
                            start=(ko == 0),
                            stop=(ko == KO - 1),
                        )
                    nc.scalar.copy(out=dsl, in_=ps)

            if only == "q":
                rope_srcs, do_v = (qc,), False
            elif only == "kv":
                rope_srcs, do_v = (kt,), True
            else:
                rope_srcs, do_v = (qc, kt), True
            # RoPE: y = x*C + roll64(x)*S' (only via PE roll + 3 DVE ops)
            for srct in rope_srcs:
                for h in range(H_LOC):
                    sl = srct[:, h, :] if srct is qc else srct[:, h, cs:cs + TCH]
                    rolled = ps_tile()
                    nc.tensor.matmul(rolled, lhsT=roll, rhs=sl,
                                     start=True, stop=True)
                    a = rtmp.tile([128, TCH], f32, tag="ra", name="ra")
                    b = rtmp.tile([128, TCH], f32, tag="rb", name="rb")
                    nc.vector.tensor_mul(out=a, in0=sl, in1=ctab[:, cs:cs + TCH])
                    nc.vector.tensor_mul(out=b, in0=rolled, in1=stab[:, cs:cs + TCH])
                    nc.vector.tensor_add(out=sl, in0=a, in1=b)
            if not do_v:
                return qc
            # split mode overlaps attention (which owns mix/ot): v uses psum_p
            vmix = mix_tile() if only is None else None
            for tt in range(TCH // 128):
                gt = c * (TCH // 128) + tt
                if vmix is not None:
                    ps = vmix[:, tt // 2,
                              (tt % 2) * HD_LOC:(tt % 2 + 1) * HD_LOC]
                else:
                    ps = ps_tile()
                for ko in range(KO):
                    nc.tensor.matmul(
                        ps[:, :HD_LOC],
                        lhsT=pieces[ko // XP][:, ko % XP, tt * 128:(tt + 1) * 128],
                        rhs=w_v[:, ko, :],
                        start=(ko == 0),
                        stop=(ko == KO - 1),
                    )
                nc.scalar.copy(out=vt[:, gt, :], in_=ps[:, :HD_LOC])

            return qc

        def attn_span(q0, W, qc, off, yc, jt_lo=0, jt_hi=None,
                      state=None):
            """Causal attention for queries [q0, q0+W), heads interleaved.

            q0 must be 128-aligned; W in {256, 512}. qc holds the chunk's
            roped queries; off is q0's offset within qc/yc."""
            d0 = q0 // 128          # first diagonal j-tile
            n_jt = d0 + W // 128
            if state is None:
                ots = [ps_tile(psum_ot) for _ in range(H_LOC)]
                vecsums = [[spool.tile([128, TCH], f32r, tag=f"vecsum{par}",
                                       name="vecsum")
                            for par in range(2)] for _ in range(H_LOC)]
            else:
                ots, vecsums = state
            if jt_hi is None:
                jt_hi = n_jt
            for jt in range(jt_lo, jt_hi):
                pair = mix_tile()
                m = jt - d0
                # diagonal block: cols < 128m fully masked -- never written,
                # never read (partial-width ops)
                lo = 128 * m if m > 0 else 0
                # score matmul skips dead columns too, but only while the
                # moving dim stays >= 256 (full fp32r rate)
                slo = lo if W - lo >= 256 else 0
                for h in range(H_LOC):
                    nc.tensor.matmul(
                        pair[:, h, slo:W],
                        lhsT=kt[:, h, jt * 128:(jt + 1) * 128],
                        rhs=qc[:, h, off + slo:off + W],
                        start=True,
                        stop=True,
                    )
                pt = ptpool.tile([128, H_LOC, TCH], f32r, tag="pt", name="pt")
                # both heads in ONE activation call (strided AP when lo > 0)
                nc.scalar.activation(out=pt[:, :, lo:W], in_=pair[:, :, lo:W],
                                     func=EXP, scale=SCALE)
                for h in range(H_LOC):
                    if m >= 0:
                        nc.vector.tensor_mul(
                            out=pt[:, h, 128 * m:128 * (m + 1)],
                            in0=pt[:, h, 128 * m:128 * (m + 1)],
                            in1=tri[:],
                        )
                    # spans starting at q0=0: jt==1 is diagonal (cols < 128
                    # unwritten), so a full-width init copy would ingest
                    # garbage -- single DVE accumulator there. Other spans
                    # split across DVE (even jt) and GPSIMD (odd jt).
                    par = jt % 2 if d0 >= 2 else 0
                    vs = vecsums[h][par]
                    eng = nc.vector if par == 0 else nc.gpsimd
                    if jt < (2 if d0 >= 2 else 1):
                        eng.tensor_copy(out=vs[:, :W], in_=pt[:, h, :W])
                    else:
                        eng.tensor_add(out=vs[:, lo:W], in0=vs[:, lo:W],
                                       in1=pt[:, h, lo:W])
                    nc.tensor.matmul(
                        ots[h][:, lo:W],
                        lhsT=vt[:, jt, h * 128:(h + 1) * 128],
                        rhs=pt[:, h, lo:W],
                        start=(jt == 0),
                        stop=(jt == n_jt - 1),
                        skip_group_check=(lo > 0),
                    )
            if jt_hi < n_jt:
                return (ots, vecsums)
            for h in range(H_LOC):
                # denominator: all-ones matmul -> column sums on all partitions
                den = mix_tile()[:, 0, :W]
                if d0 >= 2:
                    nc.tensor.matmul(den, lhsT=ones, rhs=vecsums[h][0][:, :W],
                                     start=True, stop=False)
                    nc.tensor.matmul(den, lhsT=ones, rhs=vecsums[h][1][:, :W],
                                     start=False, stop=True)
                else:
                    nc.tensor.matmul(den, lhsT=ones, rhs=vecsums[h][0][:, :W],
                                     start=True, stop=True)
                recipb = rtmp.tile([128, TCH], f32, tag="recipb", name="recipb")
                nc.vector.reciprocal(out=recipb[:, :W], in_=den)
                nc.vector.tensor_mul(out=yc[:, h, off:off + W],
                                     in0=ots[h][:, :W], in1=recipb[:, :W])

        def cproj_span(q0, W, yc, off, copy_eng=None):
            """Partial c_proj (this core's hd columns) for rows [q0, q0+W)."""
            if q0 == 0:
                nc.sync.dma_start(w_o[:], wo_r)
            for tt in range(W // 128):
                gt = q0 // 128 + tt
                for nck in range(D // 512):
                    ps = mix_tile()[:, 0, :]
                    for h in range(H_LOC):
                        nc.tensor.matmul(
                            ps,
                            lhsT=yc[:, h, off + tt * 128:off + (tt + 1) * 128],
                            rhs=w_o[:, h, nck * 512:(nck + 1) * 512],
                            start=(h == 0),
                            stop=(h == H_LOC - 1),
                        )
                    ob = opool.tile([128, 512], f32, tag="ob", name="ob")
                    if copy_eng is None:
                        nc.scalar.copy(out=ob[:], in_=ps)
                    else:
                        copy_eng.tensor_copy(out=ob[:], in_=ps)
                    nc.sync.dma_start(
                        out_d[gt * 128:(gt + 1) * 128,
                              nck * 512:(nck + 1) * 512],
                        ob[:],
                    )

        # Emission order: projections stream in chunk order; each attention
        # chunk is emitted as soon as its projections exist, EXCEPT chunk 0
        # (the smallest, 4 j-tiles) which is saved for the tail so the
        # ACT-bound final attention stretch is as short as possible.
        pieces = issue_x(0)
        for c in range(N_CH - 1):
            qc = proj_chunk(c, pieces)
            pieces = issue_x(c + 1)
            yc = ypool.tile([128, H_LOC, TCH], f32r, tag="yc", name="yc")
            attn_span(c * TCH, TCH, qc, 0, yc)
            cproj_span(c * TCH, TCH, yc, 0)
        # last chunk: q projection + rope first, then its non-diagonal
        # attention (kt/vt chunks 0..2) overlaps the k/v projections
        c = N_CH - 1
        qc = proj_chunk(c, pieces, only="q")
        yc = ypool.tile([128, H_LOC, TCH], f32r, tag="yc", name="yc")
        st = attn_span(c * TCH, TCH, qc, 0, yc, jt_hi=4 * c)
        proj_chunk(c, pieces, only="kv", qc=qc)
        attn_span(c * TCH, TCH, qc, 0, yc, jt_lo=4 * c, state=st)
        cproj_span(c * TCH, TCH, yc, 0)

    nc.compile()
    _CACHE["nc"] = nc
    return nc


def host_inputs(x, Wq, Wk, Wv, Wo):
    """Per-core input dicts (host-side shard + transpose + table prep)."""
    x2 = np.ascontiguousarray(x.reshape(T, D).T).astype(np.float32)  # (D, T)

    half = DH // 2  # 64
    af = (1.0 / 1024.0) ** np.linspace(0.0, 1.0, DH // 4, dtype=np.float32)
    af = np.concatenate([af, np.zeros(DH // 4, np.float32)])         # (64,)
    theta = np.arange(T, dtype=np.float32)[:, None] * af[None, :]    # (T, 64)
    cos = np.cos(theta).T.astype(np.float32)                         # (64, T)
    sin = np.sin(theta).T.astype(np.float32)
    ctab = np.concatenate([cos, cos], axis=0)                        # (128, T)
    stab = np.concatenate([sin, -sin], axis=0)

    roll = np.zeros((128, 128), np.float32)
    for p in range(128):
        roll[p, (p + half) % 128] = 1.0
    ones = np.ones((128, 128), np.float32)
    tri = np.triu(np.ones((128, 128), np.float32))  # tri[j, i] = i >= j

    shared = {
        "xT": x2, "ctab": ctab, "stab": stab,
        "roll": roll, "ones": ones, "tri": tri,
    }
    in_maps = []
    for c in range(N_CORES):
        sl = slice(c * HD_LOC, (c + 1) * HD_LOC)
        in_maps.append({
            **shared,
            "wqT": np.ascontiguousarray(Wq[sl, :].T),
            "wkT": np.ascontiguousarray(Wk[sl, :].T),
            "wvT": np.ascontiguousarray(Wv[sl, :].T),
            "woT": np.ascontiguousarray((Wo[:, sl] / 3.0).T),
        })
    return in_maps


def _get_runner():
    """Build the program + a persistent jitted SPMD executable (once)."""
    if "runner" in _CACHE:
        return _CACHE["runner"]

    import jax
    import concourse.mybir as mybir
    from concourse.bass2jax import (
        _bass_exec_p,
        install_neuronx_cc_hook,
        partition_id_tensor,
    )
    from jax.experimental.shard_map import shard_map
    from jax.sharding import Mesh, PartitionSpec

    nc = build_program()
    install_neuronx_cc_hook()
    assert nc.dbg_addr is None
    pid_name = nc.partition_id_tensor.name if nc.partition_id_tensor else None

    in_names, out_names, out_avals, zero_outs = [], [], [], []
    for alloc in nc.m.functions[0].allocations:
        if not isinstance(alloc, mybir.MemoryLocationSet):
            continue
        name = alloc.memorylocations[0].name
        if alloc.kind == "ExternalInput":
            if name != pid_name:
                in_names.append(name)
        elif alloc.kind == "ExternalOutput":
            out_names.append(name)
            shape = tuple(alloc.tensor_shape)
            dtype = mybir.dt.np(alloc.dtype)
            out_avals.append(jax.core.ShapedArray(shape, dtype))
            zero_outs.append(np.zeros(shape, dtype))
    n_params = len(in_names)
    all_names = list(in_names) + list(out_names)
    if pid_name is not None:
        all_names.append(pid_name)
    donate = tuple(range(n_params, n_params + len(out_names)))

    def _body(*args):
        operands = list(args)
        if pid_name is not None:
            operands.append(partition_id_tensor())
        outs = _bass_exec_p.bind(
            *operands,
            out_avals=tuple(out_avals),
            in_names=tuple(all_names),
            out_names=tuple(out_names),
            lowering_input_output_aliases=(),
            sim_require_finite=True,
            sim_require_nnan=True,
            nc=nc,
        )
        return tuple(outs)

    devices = jax.devices()[:N_CORES]
    mesh = Mesh(np.asarray(devices), ("core",))
    in_specs = (PartitionSpec("core"),) * (n_params + len(out_names))
    out_specs = (PartitionSpec("core"),) * len(out_names)
    fn = jax.jit(
        shard_map(_body, mesh=mesh, in_specs=in_specs, out_specs=out_specs,
                  check_rep=False),
        donate_argnums=donate,
        keep_unused=True,
    )
    runner = (fn, in_names, out_names, out_avals, zero_outs)
    _CACHE["runner"] = runner
    return runner


def run_spmd(in_maps):
    """Execute the SPMD program; returns per-core output dicts."""
    fn, in_names, out_names, out_avals, zero_outs = _get_runner()
    concat_in = [
        np.concatenate([np.asarray(in_maps[c][n]) for c in range(N_CORES)], axis=0)
        for n in in_names
    ]
    concat_zeros = [
        np.zeros((N_CORES * z.shape[0], *z.shape[1:]), z.dtype) for z in zero_outs
    ]
    out_arrs = fn(*concat_in, *concat_zeros)
    return [
        {n: np.asarray(out_arrs[i]).reshape(N_CORES, *out_avals[i].shape)[c]
         for i, n in enumerate(out_names)}
        for c in range(N_CORES)
    ]


def kernel(x, Wq, Wk, Wv, Wo):
    in_maps = host_inputs(np.asarray(x), np.asarray(Wq), np.asarray(Wk),
                          np.asarray(Wv), np.asarray(Wo))
    results = run_spmd(in_maps)
    out = results[0]["outp"].astype(np.float64)
    for c in range(1, N_CORES):
        out += results[c]["outp"]
    return out.astype(np.float32).reshape(1, T, D)



# revision 54
# speedup vs baseline: 1.3374x; 1.3374x over previous
"""Causal self-attention with RoPE, tensor-parallel over heads on 8 TRN2 NeuronCores.

Model (from the reference):
    q/k/v = x @ W{q,k,v}.T          x: (1, 2048, 2048), 16 heads x 128 head_dim
    rope(q), rope(k)                half-rotation, 32 nonzero freqs
    causal softmax(q k^T / sqrt(128)) @ v
    out = (y / 3) @ Wo.T

Sharding: 2 heads per core. Each core computes its heads' q/k/v projections,
attention, and a partial c_proj (its 256 columns of the hd contraction);
the host sums the 8 partial outputs (the "all-reduce after c_proj").

Per-core kernel layout choices:
  - Everything transposed so the contraction dim is always on partitions:
    host supplies x in fp8/bf16 k-subtile layouts plus pre-transposed,
    pre-quantized weight slices.
  - q/k projections in fp8e4 DoubleRow perf mode (two 128-row k-subtiles
    contracted per pass): weights pre-scaled by 64 on host (their native
    magnitude ~0.02 is subnormal in e4m3); the 64*64 factor is folded into
    the softmax exp scale.
  - v projection in bf16; everything downstream of the projections (roped
    q/k, v, attention probabilities, y, Wo) lives in fp16: DVE runs 2x on
    16-bit operands and fp16's 10-bit mantissa keeps errors ~5e-4.
  - Scores computed transposed (S^T[j, i] blocks) so the P @ V matmul needs
    no transposes.
  - Softmax without max-subtraction (scores are provably tiny: |s| < ~2),
    denominator via DVE/GPSIMD accumulation + all-ones matmuls.
  - RoPE via a PE roll matmul + 3 DVE ops, both heads fused per op.
  - Output partials in fp16 (halves the output DMA; host sums in float64).
"""

import numpy as np

T = 2048
D = 2048
H = 16
DH = 128
N_CORES = 8
H_LOC = H // N_CORES          # heads per core = 2
HD_LOC = H_LOC * DH           # local head dims = 256
TCH = 512                     # query-chunk width
N_CH = T // TCH               # 4 chunks
KO = D // 128                 # 16 contraction subtiles
KO2 = KO // 2                 # 8 DoubleRow k-subtile pairs
WS = 64.0                     # host prescale on Wq/Wk before fp8 quantization
SCALE = (DH ** 0.5) / DH      # 1/sqrt(128)

_CACHE = {}


def build_program():
    """Build (once) the single-core Bass program shared by all 8 cores."""
    if "nc" in _CACHE:
        return _CACHE["nc"]

    from contextlib import ExitStack

    import concourse.bacc as bacc
    import concourse.mybir as mybir
    import concourse.tile as tile

    f32 = mybir.dt.float32
    bf16 = mybir.dt.bfloat16
    f16 = mybir.dt.float16
    f8 = mybir.dt.float8e4
    EXP = mybir.ActivationFunctionType.Exp
    DR = mybir.MatmulPerfMode.DoubleRow

    nc = bacc.Bacc("TRN2", target_bir_lowering=False)

    x8_d = nc.dram_tensor("x8", (128, KO2, 2, T), f8, kind="ExternalInput")
    xl_d = nc.dram_tensor("x8lo", (128, KO2, 2, T), f8, kind="ExternalInput")
    wq_d = nc.dram_tensor("wq8", (128, KO2, 2, HD_LOC), f8, kind="ExternalInput")
    wk_d = nc.dram_tensor("wk8", (128, KO2, 2, HD_LOC), f8, kind="ExternalInput")
    wva_d = nc.dram_tensor("wv8a", (128, KO2, 2, HD_LOC), f8, kind="ExternalInput")
    wvb_d = nc.dram_tensor("wv8b", (128, KO2, 2, HD_LOC), f8, kind="ExternalInput")
    wvc_d = nc.dram_tensor("wv8c", (128, KO2, 2, HD_LOC), f8, kind="ExternalInput")
    wo_d = nc.dram_tensor("wo16", (HD_LOC, D), f16, kind="ExternalInput")
    ct_d = nc.dram_tensor("ctab", (128, H_LOC, T), f16, kind="ExternalInput")
    st_d = nc.dram_tensor("stab", (128, H_LOC, T), f16, kind="ExternalInput")
    roll_d = nc.dram_tensor("roll", (128, 128), f16, kind="ExternalInput")
    ones_d = nc.dram_tensor("ones", (128, 128), f16, kind="ExternalInput")
    tri_d = nc.dram_tensor("tri", (128, H_LOC, 128), f16, kind="ExternalInput")
    out_d = nc.dram_tensor("outp", (T, D), f16, kind="ExternalOutput")

    wo_r = wo_d[:].rearrange("(h p) d -> p h d", p=128)

    with tile.TileContext(nc) as tc, ExitStack() as ctx:
        persist = ctx.enter_context(tc.tile_pool(name="persist", bufs=1))
        qpool = ctx.enter_context(tc.tile_pool(name="qpool", bufs=2))
        ypool = ctx.enter_context(tc.tile_pool(name="ypool", bufs=2))
        xpool = ctx.enter_context(tc.tile_pool(name="xpool", bufs=2))
        ptpool = ctx.enter_context(tc.tile_pool(name="ptpool", bufs=3))
        rtmp = ctx.enter_context(tc.tile_pool(name="rtmp", bufs=1))
        spool = ctx.enter_context(tc.tile_pool(name="spool", bufs=2))
        opool = ctx.enter_context(tc.tile_pool(name="opool", bufs=4))
        psum_p = ctx.enter_context(tc.tile_pool(name="psum_p", bufs=2, space="PSUM"))
        psum_mix = ctx.enter_context(tc.tile_pool(name="psum_mix", bufs=2, space="PSUM"))
        psum_ot = ctx.enter_context(tc.tile_pool(name="psum_ot", bufs=2, space="PSUM"))

        def ps_tile(pool=None):
            return (pool or psum_p).tile([128, TCH], f32, tag="ps", name="ps")

        def mix_tile():
            return psum_mix.tile([128, H_LOC, TCH], f32, tag="mix", name="mix")

        # --- resident tensors ---
        w_q = persist.tile([128, KO2, 2, HD_LOC], f8, tag="w_q")
        w_k = persist.tile([128, KO2, 2, HD_LOC], f8, tag="w_k")
        w_va = persist.tile([128, KO2, 2, HD_LOC], f8, tag="w_va")
        w_vb = persist.tile([128, KO2, 2, HD_LOC], f8, tag="w_vb")
        w_vc = persist.tile([128, KO2, 2, HD_LOC], f8, tag="w_vc")
        w_o = persist.tile([128, H_LOC, D], f16, tag="w_o")
        kt = persist.tile([128, H_LOC, T], f16, tag="kt")
        vt = persist.tile([128, KO, HD_LOC], f16, tag="vt")
        ctab = persist.tile([128, H_LOC, T], f16, tag="ctab")
        stab = persist.tile([128, H_LOC, T], f16, tag="stab")
        roll = persist.tile([128, 128], f16, tag="roll")
        ones = persist.tile([128, 128], f16, tag="ones")
        tri = persist.tile([128, H_LOC, 128], f16, tag="tri")

        def issue_x(c):
            """Queue the x chunk DMAs for chunk c (weights too on chunk 0)."""
            cs = c * TCH
            x8c = xpool.tile([128, KO2, 2, TCH], f8, tag="x8c", name="x8c")
            xloc = xpool.tile([128, KO2, 2, TCH], f8, tag="xloc", name="xloc")
            if c == 0:
                # ordered so the first PE work (q-proj, rope) unblocks soonest
                nc.sync.dma_start(x8c[:], x8_d[:, :, :, cs:cs + TCH])
                nc.sync.dma_start(w_q[:], wq_d[:])
                nc.sync.dma_start(roll[:], roll_d[:])
                nc.sync.dma_start(ctab[:], ct_d[:])
                nc.sync.dma_start(stab[:], st_d[:])
                nc.sync.dma_start(w_k[:], wk_d[:])
                nc.sync.dma_start(w_va[:], wva_d[:])
                nc.sync.dma_start(xloc[:], xl_d[:, :, :, cs:cs + TCH])
                nc.sync.dma_start(w_vb[:], wvb_d[:])
                nc.sync.dma_start(w_vc[:], wvc_d[:])
                nc.sync.dma_start(tri[:], tri_d[:])
                nc.sync.dma_start(ones[:], ones_d[:])
                nc.sync.dma_start(w_o[:], wo_r)
            else:
                nc.sync.dma_start(x8c[:], x8_d[:, :, :, cs:cs + TCH])
                nc.sync.dma_start(xloc[:], xl_d[:, :, :, cs:cs + TCH])
            return (x8c, xloc)

        def rope(sl, cs):
            """RoPE in place: y = x*C + roll64(x)*S', heads fused per DVE op."""
            rolled = mix_tile()
            for h in range(H_LOC):
                nc.tensor.matmul(rolled[:, h, :], lhsT=roll, rhs=sl[:, h, :],
                                 start=True, stop=True)
            a = rtmp.tile([128, H_LOC, TCH], f16, tag="ra", name="ra")
            b = rtmp.tile([128, H_LOC, TCH], f16, tag="rb", name="rb")
            nc.vector.tensor_mul(out=a, in0=sl, in1=ctab[:, :, cs:cs + TCH])
            nc.vector.tensor_mul(out=b, in0=rolled, in1=stab[:, :, cs:cs + TCH])
            nc.vector.tensor_add(out=sl, in0=a, in1=b)

        def qk_head(w_sb, x8c, dsl, h):
            """One head's q-or-k projection: 16 DoubleRow matmuls + copy."""
            ps = ps_tile()
            for tp in range(2):
                for jko in range(KO2):
                    nc.tensor.matmul(
                        ps[:, tp * 256:(tp + 1) * 256],
                        lhsT=w_sb[:, jko, :, h * 128:(h + 1) * 128],
                        rhs=x8c[:, jko, :, tp * 256:(tp + 1) * 256],
                        start=(jko == 0),
                        stop=(jko == KO2 - 1),
                        perf_mode=DR,
                    )
            nc.scalar.copy(out=dsl, in_=ps)

        def v_half(c, xc, tu):
            """Half a chunk's v projection: 3-pass split-fp8 DoubleRow.

            v = x8*Wva(64w) + xlo(8dx)*Wvb(8w) + x8*Wvc(64dw); PSUM holds
            64*v, the evacuation copy scales by 1/64."""
            x8c, xloc = xc
            passes = ((x8c, w_va), (xloc, w_vb), (x8c, w_vc))
            ps = ps_tile()
            for tt in (2 * tu, 2 * tu + 1):
                sub = ps[:, (tt % 2) * HD_LOC:(tt % 2 + 1) * HD_LOC]
                for pi, (xt, wt) in enumerate(passes):
                    for jko in range(KO2):
                        nc.tensor.matmul(
                            sub,
                            lhsT=xt[:, jko, :, tt * 128:(tt + 1) * 128],
                            rhs=wt[:, jko, :, :],
                            start=(pi == 0 and jko == 0),
                            stop=(pi == 2 and jko == KO2 - 1),
                            perf_mode=DR,
                        )
            gt2 = c * 2 + tu
            nc.scalar.mul(out=vt[:, 2 * gt2:2 * gt2 + 2, :], in_=ps,
                          mul=1.0 / WS)

        def proj_q(c, xc, qtag="qc"):
            """q projection + its rope for t-chunk c."""
            qc = qpool.tile([128, H_LOC, TCH], f16, tag=qtag, name="qc")
            for h in range(H_LOC):
                qk_head(w_q, xc[0], qc[:, h, :], h)
            rope(qc[:, :, :], c * TCH)
            return qc

        def kv_quanta(c, xc):
            """k/v projections for chunk c as quanta (PE-heavy, ACT-light) --
            interleaved into the previous chunk's attention span."""
            cs = c * TCH

            def k_head(h):
                qk_head(w_k, xc[0], kt[:, h, cs:cs + TCH], h)

            return [
                lambda: k_head(0),
                lambda: k_head(1),
                lambda: rope(kt[:, :, cs:cs + TCH], cs),
                lambda: v_half(c, xc, 0),
                lambda: v_half(c, xc, 1),
            ]

        def attn_span(q0, W, qc, off, yc, jt_lo=0, jt_hi=None,
                      state=None, ot_pool=None, filler=(), fill_per_jt=1):
            """Causal attention for queries [q0, q0+W), heads interleaved.

            q0 must be 128-aligned; W in {256, 512}. qc holds the chunk's
            roped queries; off is q0's offset within qc/yc."""
            d0 = q0 // 128          # first diagonal j-tile
            n_jt = d0 + W // 128
            if state is None:
                ots = [ps_tile(ot_pool or psum_ot) for _ in range(H_LOC)]
                vecsums = [spool.tile([128, H_LOC, TCH], f16,
                                      tag=f"vecsum{par}", name="vecsum")
                           for par in range(2)]
            else:
                ots, vecsums = state
            if jt_hi is None:
                jt_hi = n_jt
            filler = iter(filler) if not hasattr(filler, "__next__") else filler
            for jt in range(jt_lo, jt_hi):
                # interleave deferred work (previous chunk's c_proj) into the
                # jt loop: PE's stream is in-order per engine, so this is the
                # only way it can fill the exp-gated gaps between j-tiles
                for _ in range(fill_per_jt):
                    q = next(filler, None)
                    if q is not None:
                        q()
                pair = mix_tile()
                m = jt - d0
                # diagonal block: cols < 128m fully masked -- never written,
                # never read (partial-width ops)
                lo = 128 * m if m > 0 else 0
                # score matmul skips dead columns too, but only while the
                # moving dim stays >= 256 (full rate)
                slo = lo if W - lo >= 256 else 0
                for h in range(H_LOC):
                    nc.tensor.matmul(
                        pair[:, h, slo:W],
                        lhsT=kt[:, h, jt * 128:(jt + 1) * 128],
                        rhs=qc[:, h, off + slo:off + W],
                        start=True,
                        stop=True,
                    )
                pt = ptpool.tile([128, H_LOC, TCH], f16, tag="pt", name="pt")
                # both heads in ONE activation call (strided AP when lo > 0);
                # q/k carry the 64x host prescale each -> 1/4096 here
                nc.scalar.activation(out=pt[:, :, lo:W], in_=pair[:, :, lo:W],
                                     func=EXP, scale=SCALE / (WS * WS))
                if m >= 0:
                    # mask the diagonal block, both heads in one op
                    nc.vector.tensor_mul(
                        out=pt[:, :, 128 * m:128 * (m + 1)],
                        in0=pt[:, :, 128 * m:128 * (m + 1)],
                        in1=tri[:],
                    )
                # probability row-sum accumulator (all DVE: f16 runs 2x and
                # GPSIMD's 0.42-efficiency adds would chain on the critical
                # path). jt==0 initializes via copy; on q0=0 spans jt==1 is
                # diagonal with cols < 128 unwritten, so never full-copy there.
                vs = vecsums[0]
                if jt == 0:
                    nc.vector.tensor_copy(out=vs[:, :, :W], in_=pt[:, :, :W])
                else:
                    nc.vector.tensor_add(out=vs[:, :, lo:W], in0=vs[:, :, lo:W],
                                         in1=pt[:, :, lo:W])
                for h in range(H_LOC):
                    # partial-width diagonal writes skip the (bank-granular)
                    # psum group check -- EXCEPT the last j-tile, whose
                    # stop must be bookkept so the ymul read sees a closed
                    # group
                    nc.tensor.matmul(
                        ots[h][:, lo:W],
                        lhsT=vt[:, jt, h * 128:(h + 1) * 128],
                        rhs=pt[:, h, lo:W],
                        start=(jt == 0),
                        stop=(jt == n_jt - 1),
                        skip_group_check=(lo > 0 and jt != n_jt - 1),
                    )
            if jt_hi < n_jt:
                return (ots, vecsums), filler
            # denominator: all-ones matmuls -> column sums on all partitions;
            # one psum tile + one fused reciprocal for both heads
            den = mix_tile()
            for h in range(H_LOC):
                nc.tensor.matmul(den[:, h, :W], lhsT=ones,
                                 rhs=vecsums[0][:, h, :W],
                                 start=True, stop=True)
            recipb = rtmp.tile([128, H_LOC, TCH], f32, tag="recipb",
                               name="recipb")
            nc.vector.reciprocal(out=recipb[:, :, :W], in_=den[:, :, :W])
            for h in range(H_LOC):
                nc.vector.tensor_mul(out=yc[:, h, off:off + W],
                                     in0=ots[h][:, :W], in1=recipb[:, h, :W])
            return filler

        def cproj_quanta(q0, W, yc, off):
            """Partial c_proj for rows [q0, q0+W) as a list of work quanta.

            Each quantum emits half a 128-row tile (4 matmuls + one [128,
            1024] PSUM evacuation + its output DMA); the caller threads them
            into an attention span's jt loop so PE fills exp-gated gaps."""
            obs = {}

            def quantum(tt, half):
                gt = q0 // 128 + tt
                if half == 0:
                    obs[tt] = opool.tile([128, D], f16, tag="ob", name="ob")
                ps = mix_tile()
                for nk in range(2):
                    nck = half * 2 + nk
                    for h in range(H_LOC):
                        nc.tensor.matmul(
                            ps[:, nk, :],
                            lhsT=yc[:, h,
                                    off + tt * 128:off + (tt + 1) * 128],
                            rhs=w_o[:, h, nck * 512:(nck + 1) * 512],
                            start=(h == 0),
                            stop=(h == H_LOC - 1),
                        )
                # evacuate [128, 1024] in one instr; alternate ACT/DVE,
                # DMA each half as soon as its copy lands
                osl = obs[tt][:, half * 1024:(half + 1) * 1024]
                if half == 0:
                    nc.scalar.copy(out=osl, in_=ps)
                else:
                    nc.vector.tensor_copy(out=osl, in_=ps)
                nc.sync.dma_start(
                    out_d[gt * 128:(gt + 1) * 128,
                          half * 1024:(half + 1) * 1024],
                    osl,
                )

            return [
                (lambda tt=tt, half=half: quantum(tt, half))
                for tt in range(W // 128) for half in range(2)
            ]

        def drain(filler):
            filler = iter(filler) if not hasattr(filler, "__next__") else filler
            for q in filler:
                q()

        # Emission order: projections stream in chunk order; each attention
        # chunk is emitted as soon as its projections exist. The last chunk
        # splits q from k/v so its early j-tiles overlap the projections, and
        # chunk 0's (tiny) attention is saved for the very end so the serial
        # tail after the final projections is as short as possible.
        xc0 = issue_x(0)
        qc0 = proj_q(0, xc0, qtag="qc0")
        drain(kv_quanta(0, xc0))
        yc0 = ypool.tile([128, H_LOC, TCH], f16, tag="yc0", name="yc0")
        xc1 = issue_x(1)
        qc1 = proj_q(1, xc1)
        drain(kv_quanta(1, xc1))
        yc1 = ypool.tile([128, H_LOC, TCH], f16, tag="yc", name="yc")
        xc2 = issue_x(2)
        # chunk c+1's k/v projections interleave into chunk c's attention:
        # they are PE-dense but ACT/DVE-light, exactly what the exp-gated
        # jt loop can absorb
        attn_span(TCH, TCH, qc1, 0, yc1, filler=kv_quanta(2, xc2))
        qc2 = proj_q(2, xc2)
        drain(cproj_quanta(TCH, TCH, yc1, 0))
        yc2 = ypool.tile([128, H_LOC, TCH], f16, tag="yc", name="yc")
        xc3 = issue_x(3)
        attn_span(2 * TCH, TCH, qc2, 0, yc2, filler=kv_quanta(3, xc3))
        qc3 = proj_q(3, xc3)
        drain(cproj_quanta(2 * TCH, TCH, yc2, 0))
        yc3 = ypool.tile([128, H_LOC, TCH], f16, tag="yc", name="yc")
        attn_span(3 * TCH, TCH, qc3, 0, yc3)
        # chunk 0's attention runs under cproj3: projections are done by now,
        # so its PV accumulators can live in the idle proj psum ring
        attn_span(0, TCH, qc0, 0, yc0, ot_pool=psum_p)
        drain(cproj_quanta(3 * TCH, TCH, yc3, 0))
        drain(cproj_quanta(0, TCH, yc0, 0))

    nc.compile()
    _CACHE["nc"] = nc
    return nc


def host_inputs(x, Wq, Wk, Wv, Wo):
    """Per-core input dicts (host-side shard + transpose + quantize + tables)."""
    import ml_dtypes

    f8 = ml_dtypes.float8_e4m3
    bf16 = ml_dtypes.bfloat16

    def pack_x8(a):  # (D, T) f32 -> (128, KO2, 2, T) fp8 DoubleRow layout
        return np.ascontiguousarray(
            a.reshape(KO2, 2, 128, T).transpose(2, 0, 1, 3)).astype(f8)

    x2 = np.ascontiguousarray(x.reshape(T, D).T).astype(np.float32)  # (D, T)
    x8 = pack_x8(x2)
    # fp8 residual (scaled 8x) for the v projection's second pass
    x8lo = pack_x8(
        8.0 * (x2 - x8.transpose(1, 2, 0, 3).reshape(D, T).astype(np.float32)))

    half = DH // 2  # 64
    af = (1.0 / 1024.0) ** np.linspace(0.0, 1.0, DH // 4, dtype=np.float32)
    af = np.concatenate([af, np.zeros(DH // 4, np.float32)])         # (64,)
    theta = np.arange(T, dtype=np.float32)[:, None] * af[None, :]    # (T, 64)
    cos = np.cos(theta).T.astype(np.float32)                         # (64, T)
    sin = np.sin(theta).T.astype(np.float32)
    ctab1 = np.concatenate([cos, cos], axis=0)                       # (128, T)
    stab1 = np.concatenate([sin, -sin], axis=0)
    # duplicated per head for head-fused rope ops: (128, H_LOC, T)
    ctab = np.repeat(ctab1[:, None, :], H_LOC, axis=1).astype(np.float16)
    stab = np.repeat(stab1[:, None, :], H_LOC, axis=1).astype(np.float32)

    roll = np.zeros((128, 128), np.float16)
    for p in range(128):
        roll[p, (p + half) % 128] = 1.0
    ones = np.ones((128, 128), np.float16)
    tri1 = np.triu(np.ones((128, 128), np.float16))  # tri[j, i] = i >= j
    tri = np.repeat(tri1[:, None, :], H_LOC, axis=1)

    shared = {
        "x8": x8, "x8lo": x8lo, "ctab": ctab, "stab": stab,
        "roll": roll, "ones": ones, "tri": tri,
    }

    def pack_pre(wt):  # pre-scaled (D, HD_LOC) f32 -> DoubleRow fp8 layout
        return np.ascontiguousarray(
            wt.reshape(KO2, 2, 128, HD_LOC).transpose(2, 0, 1, 3)).astype(f8)

    def pack_w8(w):  # (HD_LOC, D) slice -> (128, KO2, 2, HD_LOC) fp8, x WS
        return pack_pre((w.T * WS).astype(np.float32))

    in_maps = []
    for c in range(N_CORES):
        sl = slice(c * HD_LOC, (c + 1) * HD_LOC)
        wv_t = Wv[sl, :].T.astype(np.float32)               # (D, HD_LOC)
        wv8a = pack_pre(wv_t * WS)
        # residual of the 64x-quantized Wv, itself scaled 64x
        wv_res = wv_t - wv8a.transpose(1, 2, 0, 3).reshape(D, HD_LOC).astype(
            np.float32) / WS
        in_maps.append({
            **shared,
            "wq8": pack_w8(Wq[sl, :]),
            "wk8": pack_w8(Wk[sl, :]),
            "wv8a": wv8a,
            "wv8b": pack_pre(wv_t * 8.0),
            "wv8c": pack_pre(wv_res * WS),
            "wo16": np.ascontiguousarray((Wo[:, sl] / 3.0).T).astype(np.float16),
        })
    return in_maps


def _get_runner():
    """Build the program + a persistent jitted SPMD executable (once)."""
    if "runner" in _CACHE:
        return _CACHE["runner"]

    import jax
    import concourse.mybir as mybir
    from concourse.bass2jax import (
        _bass_exec_p,
        install_neuronx_cc_hook,
        partition_id_tensor,
    )
    from jax.experimental.shard_map import shard_map
    from jax.sharding import Mesh, PartitionSpec

    nc = build_program()
    install_neuronx_cc_hook()
    assert nc.dbg_addr is None
    pid_name = nc.partition_id_tensor.name if nc.partition_id_tensor else None

    in_names, out_names, out_avals, zero_outs = [], [], [], []
    for alloc in nc.m.functions[0].allocations:
        if not isinstance(alloc, mybir.MemoryLocationSet):
            continue
        name = alloc.memorylocations[0].name
        if alloc.kind == "ExternalInput":
            if name != pid_name:
                in_names.append(name)
        elif alloc.kind == "ExternalOutput":
            out_names.append(name)
            shape = tuple(alloc.tensor_shape)
            dtype = mybir.dt.np(alloc.dtype)
            out_avals.append(jax.core.ShapedArray(shape, dtype))
            zero_outs.append(np.zeros(shape, dtype))
    n_params = len(in_names)
    all_names = list(in_names) + list(out_names)
    if pid_name is not None:
        all_names.append(pid_name)
    donate = tuple(range(n_params, n_params + len(out_names)))

    def _body(*args):
        operands = list(args)
        if pid_name is not None:
            operands.append(partition_id_tensor())
        outs = _bass_exec_p.bind(
            *operands,
            out_avals=tuple(out_avals),
            in_names=tuple(all_names),
            out_names=tuple(out_names),
            lowering_input_output_aliases=(),
            sim_require_finite=True,
            sim_require_nnan=True,
            nc=nc,
        )
        return tuple(outs)

    devices = jax.devices()[:N_CORES]
    mesh = Mesh(np.asarray(devices), ("core",))
    in_specs = (PartitionSpec("core"),) * (n_params + len(out_names))
    out_specs = (PartitionSpec("core"),) * len(out_names)
    fn = jax.jit(
        shard_map(_body, mesh=mesh, in_specs=in_specs, out_specs=out_specs,
                  check_rep=False),
        donate_argnums=donate,
        keep_unused=True,
    )
    runner = (fn, in_names, out_names, out_avals, zero_outs)
    _CACHE["runner"] = runner
    return runner


def run_spmd(in_maps):
    """Execute the SPMD program; returns per-core output dicts."""
    fn, in_names, out_names, out_avals, zero_outs = _get_runner()
    concat_in = [
        np.concatenate([np.asarray(in_maps[c][n]) for c in range(N_CORES)], axis=0)
        for n in in_names
    ]
    concat_zeros = [
        np.zeros((N_CORES * z.shape[0], *z.shape[1:]), z.dtype) for z in zero_outs
    ]
    out_arrs = fn(*concat_in, *concat_zeros)
    return [
        {n: np.asarray(out_arrs[i]).reshape(N_CORES, *out_avals[i].shape)[c]
         for i, n in enumerate(out_names)}
        for c in range(N_CORES)
    ]


def kernel(x, Wq, Wk, Wv, Wo):
    in_maps = host_inputs(np.asarray(x), np.asarray(Wq), np.asarray(Wk),
                          np.asarray(Wv), np.asarray(Wo))
    results = run_spmd(in_maps)
    out = results[0]["outp"].astype(np.float64)
    for c in range(1, N_CORES):
        out += results[c]["outp"].astype(np.float64)
    return out.astype(np.float32).reshape(1, T, D)


# revision 55
# speedup vs baseline: 1.3852x; 1.0357x over previous
"""Causal self-attention with RoPE, tensor-parallel over heads on 8 TRN2 NeuronCores.

Model (from the reference):
    q/k/v = x @ W{q,k,v}.T          x: (1, 2048, 2048), 16 heads x 128 head_dim
    rope(q), rope(k)                half-rotation, 32 nonzero freqs
    causal softmax(q k^T / sqrt(128)) @ v
    out = (y / 3) @ Wo.T

Sharding: 2 heads per core. Each core computes its heads' q/k/v projections,
attention, and a partial c_proj (its 256 columns of the hd contraction);
the host sums the 8 partial outputs (the "all-reduce after c_proj").

Per-core kernel layout choices:
  - Everything transposed so the contraction dim is always on partitions:
    host supplies x in fp8/bf16 k-subtile layouts plus pre-transposed,
    pre-quantized weight slices.
  - q/k projections in fp8e4 DoubleRow perf mode (two 128-row k-subtiles
    contracted per pass): weights pre-scaled by 64 on host (their native
    magnitude ~0.02 is subnormal in e4m3); the 64*64 factor is folded into
    the softmax exp scale.
  - v projection in bf16; everything downstream of the projections (roped
    q/k, v, attention probabilities, y, Wo) lives in fp16: DVE runs 2x on
    16-bit operands and fp16's 10-bit mantissa keeps errors ~5e-4.
  - Scores computed transposed (S^T[j, i] blocks) so the P @ V matmul needs
    no transposes.
  - Softmax without max-subtraction (scores are provably tiny: |s| < ~2),
    denominator via DVE/GPSIMD accumulation + all-ones matmuls.
  - RoPE via a PE roll matmul + 3 DVE ops, both heads fused per op.
  - Output partials in fp16 (halves the output DMA; host sums in float64).
"""

import numpy as np

T = 2048
D = 2048
H = 16
DH = 128
N_CORES = 8
H_LOC = H // N_CORES          # heads per core = 2
HD_LOC = H_LOC * DH           # local head dims = 256
TCH = 512                     # query-chunk width
N_CH = T // TCH               # 4 chunks
KO = D // 128                 # 16 contraction subtiles
KO2 = KO // 2                 # 8 DoubleRow k-subtile pairs
WS = 64.0                     # host prescale on Wq/Wk before fp8 quantization
SCALE = (DH ** 0.5) / DH      # 1/sqrt(128)

_CACHE = {}


def build_program():
    """Build (once) the single-core Bass program shared by all 8 cores."""
    if "nc" in _CACHE:
        return _CACHE["nc"]

    from contextlib import ExitStack

    import concourse.bacc as bacc
    import concourse.mybir as mybir
    import concourse.tile as tile

    f32 = mybir.dt.float32
    bf16 = mybir.dt.bfloat16
    f16 = mybir.dt.float16
    f8 = mybir.dt.float8e4
    EXP = mybir.ActivationFunctionType.Exp
    DR = mybir.MatmulPerfMode.DoubleRow

    nc = bacc.Bacc("TRN2", target_bir_lowering=False)

    x8_d = nc.dram_tensor("x8", (128, KO2, 2, T), f8, kind="ExternalInput")
    xl_d = nc.dram_tensor("x8lo", (128, KO2, 2, T), f8, kind="ExternalInput")
    wq_d = nc.dram_tensor("wq8", (128, KO2, 2, HD_LOC), f8, kind="ExternalInput")
    wk_d = nc.dram_tensor("wk8", (128, KO2, 2, HD_LOC), f8, kind="ExternalInput")
    wva_d = nc.dram_tensor("wv8a", (128, KO2, 2, HD_LOC), f8, kind="ExternalInput")
    wvb_d = nc.dram_tensor("wv8b", (128, KO2, 2, HD_LOC), f8, kind="ExternalInput")
    wvc_d = nc.dram_tensor("wv8c", (128, KO2, 2, HD_LOC), f8, kind="ExternalInput")
    wo_d = nc.dram_tensor("wo16", (HD_LOC, D), f16, kind="ExternalInput")
    ct_d = nc.dram_tensor("ctab", (128, H_LOC, T), f16, kind="ExternalInput")
    st_d = nc.dram_tensor("stab", (128, H_LOC, T), f16, kind="ExternalInput")
    roll_d = nc.dram_tensor("roll", (128, 128), f16, kind="ExternalInput")
    ones_d = nc.dram_tensor("ones", (128, 128), f16, kind="ExternalInput")
    tri_d = nc.dram_tensor("tri", (128, H_LOC, 128), f16, kind="ExternalInput")
    out_d = nc.dram_tensor("outp", (T, D), f16, kind="ExternalOutput")

    wo_r = wo_d[:].rearrange("(h p) d -> p h d", p=128)

    with tile.TileContext(nc) as tc, ExitStack() as ctx:
        persist = ctx.enter_context(tc.tile_pool(name="persist", bufs=1))
        qpool = ctx.enter_context(tc.tile_pool(name="qpool", bufs=2))
        ypool = ctx.enter_context(tc.tile_pool(name="ypool", bufs=2))
        xpool = ctx.enter_context(tc.tile_pool(name="xpool", bufs=2))
        ptpool = ctx.enter_context(tc.tile_pool(name="ptpool", bufs=4))
        rtmp = ctx.enter_context(tc.tile_pool(name="rtmp", bufs=1))
        spool = ctx.enter_context(tc.tile_pool(name="spool", bufs=2))
        opool = ctx.enter_context(tc.tile_pool(name="opool", bufs=6))
        psum_p = ctx.enter_context(tc.tile_pool(name="psum_p", bufs=2, space="PSUM"))
        psum_mix = ctx.enter_context(tc.tile_pool(name="psum_mix", bufs=2, space="PSUM"))
        psum_ot = ctx.enter_context(tc.tile_pool(name="psum_ot", bufs=2, space="PSUM"))

        def ps_tile(pool=None):
            return (pool or psum_p).tile([128, TCH], f32, tag="ps", name="ps")

        def mix_tile():
            return psum_mix.tile([128, H_LOC, TCH], f32, tag="mix", name="mix")

        # --- resident tensors ---
        w_q = persist.tile([128, KO2, 2, HD_LOC], f8, tag="w_q")
        w_k = persist.tile([128, KO2, 2, HD_LOC], f8, tag="w_k")
        w_va = persist.tile([128, KO2, 2, HD_LOC], f8, tag="w_va")
        w_vb = persist.tile([128, KO2, 2, HD_LOC], f8, tag="w_vb")
        w_vc = persist.tile([128, KO2, 2, HD_LOC], f8, tag="w_vc")
        w_o = persist.tile([128, H_LOC, D], f16, tag="w_o")
        kt = persist.tile([128, H_LOC, T], f16, tag="kt")
        vt = persist.tile([128, KO, HD_LOC], f16, tag="vt")
        ctab = persist.tile([128, H_LOC, T], f16, tag="ctab")
        stab = persist.tile([128, H_LOC, T], f16, tag="stab")
        roll = persist.tile([128, 128], f16, tag="roll")
        ones = persist.tile([128, 128], f16, tag="ones")
        tri = persist.tile([128, H_LOC, 128], f16, tag="tri")

        def issue_x(c):
            """Queue the x chunk DMAs for chunk c (weights too on chunk 0)."""
            cs = c * TCH
            x8c = xpool.tile([128, KO2, 2, TCH], f8, tag="x8c", name="x8c")
            xloc = xpool.tile([128, KO2, 2, TCH], f8, tag="xloc", name="xloc")
            if c == 0:
                # ordered so the first PE work (q-proj, rope) unblocks
                # soonest; first transfers split+interleaved so the opening
                # matmul group can start after half the data has landed
                nc.sync.dma_start(x8c[:, :4], x8_d[:, :4, :, cs:cs + TCH])
                nc.sync.dma_start(w_q[:, :4], wq_d[:, :4])
                nc.sync.dma_start(x8c[:, 4:], x8_d[:, 4:, :, cs:cs + TCH])
                nc.sync.dma_start(w_q[:, 4:], wq_d[:, 4:])
                nc.sync.dma_start(w_k[:], wk_d[:])
                nc.sync.dma_start(roll[:], roll_d[:])
                nc.sync.dma_start(ctab[:], ct_d[:])
                nc.sync.dma_start(stab[:], st_d[:])
                nc.sync.dma_start(w_va[:], wva_d[:])
                nc.sync.dma_start(xloc[:], xl_d[:, :, :, cs:cs + TCH])
                nc.sync.dma_start(w_vb[:], wvb_d[:])
                nc.sync.dma_start(w_vc[:], wvc_d[:])
                nc.sync.dma_start(tri[:], tri_d[:])
                nc.sync.dma_start(ones[:], ones_d[:])
                nc.sync.dma_start(w_o[:], wo_r)
            else:
                nc.sync.dma_start(x8c[:], x8_d[:, :, :, cs:cs + TCH])
                nc.sync.dma_start(xloc[:], xl_d[:, :, :, cs:cs + TCH])
            return (x8c, xloc)

        def rope(sl, cs):
            """RoPE in place: y = x*C + roll64(x)*S', heads fused per DVE op."""
            rolled = mix_tile()
            for h in range(H_LOC):
                nc.tensor.matmul(rolled[:, h, :], lhsT=roll, rhs=sl[:, h, :],
                                 start=True, stop=True)
            a = rtmp.tile([128, H_LOC, TCH], f16, tag="ra", name="ra")
            b = rtmp.tile([128, H_LOC, TCH], f16, tag="rb", name="rb")
            nc.vector.tensor_mul(out=a, in0=sl, in1=ctab[:, :, cs:cs + TCH])
            nc.vector.tensor_mul(out=b, in0=rolled, in1=stab[:, :, cs:cs + TCH])
            nc.vector.tensor_add(out=sl, in0=a, in1=b)

        def qk_head(w_sb, x8c, dsl, h):
            """One head's q-or-k projection: 16 DoubleRow matmuls + copy."""
            ps = ps_tile()
            for tp in range(2):
                for jko in range(KO2):
                    nc.tensor.matmul(
                        ps[:, tp * 256:(tp + 1) * 256],
                        lhsT=w_sb[:, jko, :, h * 128:(h + 1) * 128],
                        rhs=x8c[:, jko, :, tp * 256:(tp + 1) * 256],
                        start=(jko == 0),
                        stop=(jko == KO2 - 1),
                        perf_mode=DR,
                    )
            nc.scalar.copy(out=dsl, in_=ps)

        def v_half(c, xc, tu):
            """Half a chunk's v projection: 3-pass split-fp8 DoubleRow.

            v = x8*Wva(64w) + xlo(8dx)*Wvb(8w) + x8*Wvc(64dw); PSUM holds
            64*v, the evacuation copy scales by 1/64."""
            x8c, xloc = xc
            passes = ((x8c, w_va), (xloc, w_vb), (x8c, w_vc))
            ps = ps_tile()
            for tt in (2 * tu, 2 * tu + 1):
                sub = ps[:, (tt % 2) * HD_LOC:(tt % 2 + 1) * HD_LOC]
                for pi, (xt, wt) in enumerate(passes):
                    for jko in range(KO2):
                        nc.tensor.matmul(
                            sub,
                            lhsT=xt[:, jko, :, tt * 128:(tt + 1) * 128],
                            rhs=wt[:, jko, :, :],
                            start=(pi == 0 and jko == 0),
                            stop=(pi == 2 and jko == KO2 - 1),
                            perf_mode=DR,
                        )
            gt2 = c * 2 + tu
            nc.scalar.mul(out=vt[:, 2 * gt2:2 * gt2 + 2, :], in_=ps,
                          mul=1.0 / WS)

        def proj_q(c, xc, qtag="qc"):
            """q projection + its rope for t-chunk c."""
            qc = qpool.tile([128, H_LOC, TCH], f16, tag=qtag, name="qc")
            for h in range(H_LOC):
                qk_head(w_q, xc[0], qc[:, h, :], h)
            rope(qc[:, :, :], c * TCH)
            return qc

        def kv_quanta(c, xc):
            """k/v projections for chunk c as quanta (PE-heavy, ACT-light) --
            interleaved into the previous chunk's attention span."""
            cs = c * TCH

            def k_head(h):
                qk_head(w_k, xc[0], kt[:, h, cs:cs + TCH], h)

            return [
                lambda: k_head(0),
                lambda: k_head(1),
                lambda: rope(kt[:, :, cs:cs + TCH], cs),
                lambda: v_half(c, xc, 0),
                lambda: v_half(c, xc, 1),
            ]

        def attn_span(q0, W, qc, off, yc, jt_lo=0, jt_hi=None,
                      state=None, ot_pool=None, filler=(), fill_per_jt=1):
            """Causal attention for queries [q0, q0+W), heads interleaved.

            q0 must be 128-aligned; W in {256, 512}. qc holds the chunk's
            roped queries; off is q0's offset within qc/yc."""
            d0 = q0 // 128          # first diagonal j-tile
            n_jt = d0 + W // 128
            if state is None:
                ots = [ps_tile(ot_pool or psum_ot) for _ in range(H_LOC)]
                vecsums = [spool.tile([128, H_LOC, TCH], f16,
                                      tag=f"vecsum{par}", name="vecsum")
                           for par in range(2)]
            else:
                ots, vecsums = state
            if jt_hi is None:
                jt_hi = n_jt
            filler = iter(filler) if not hasattr(filler, "__next__") else filler
            for jt in range(jt_lo, jt_hi):
                # interleave deferred work (previous chunk's c_proj) into the
                # jt loop: PE's stream is in-order per engine, so this is the
                # only way it can fill the exp-gated gaps between j-tiles
                for _ in range(fill_per_jt):
                    q = next(filler, None)
                    if q is not None:
                        q()
                pair = mix_tile()
                m = jt - d0
                # diagonal block: cols < 128m fully masked -- never written,
                # never read (partial-width ops)
                lo = 128 * m if m > 0 else 0
                # score matmul skips dead columns too, but only while the
                # moving dim stays >= 256 (full rate)
                slo = lo if W - lo >= 256 else 0
                for h in range(H_LOC):
                    nc.tensor.matmul(
                        pair[:, h, slo:W],
                        lhsT=kt[:, h, jt * 128:(jt + 1) * 128],
                        rhs=qc[:, h, off + slo:off + W],
                        start=True,
                        stop=True,
                    )
                pt = ptpool.tile([128, H_LOC, TCH], f16, tag="pt", name="pt")
                # both heads in ONE activation call (strided AP when lo > 0);
                # q/k carry the 64x host prescale each -> 1/4096 here
                nc.scalar.activation(out=pt[:, :, lo:W], in_=pair[:, :, lo:W],
                                     func=EXP, scale=SCALE / (WS * WS))
                if m >= 0:
                    # mask the diagonal block, both heads in one op
                    nc.vector.tensor_mul(
                        out=pt[:, :, 128 * m:128 * (m + 1)],
                        in0=pt[:, :, 128 * m:128 * (m + 1)],
                        in1=tri[:],
                    )
                # probability row-sum accumulator (all DVE: f16 runs 2x and
                # GPSIMD's 0.42-efficiency adds would chain on the critical
                # path). jt==0 initializes via copy; on q0=0 spans jt==1 is
                # diagonal with cols < 128 unwritten, so never full-copy there.
                vs = vecsums[0]
                if jt == 0:
                    nc.vector.tensor_copy(out=vs[:, :, :W], in_=pt[:, :, :W])
                else:
                    nc.vector.tensor_add(out=vs[:, :, lo:W], in0=vs[:, :, lo:W],
                                         in1=pt[:, :, lo:W])
                for h in range(H_LOC):
                    # partial-width diagonal writes skip the (bank-granular)
                    # psum group check -- EXCEPT the last j-tile, whose
                    # stop must be bookkept so the ymul read sees a closed
                    # group
                    nc.tensor.matmul(
                        ots[h][:, lo:W],
                        lhsT=vt[:, jt, h * 128:(h + 1) * 128],
                        rhs=pt[:, h, lo:W],
                        start=(jt == 0),
                        stop=(jt == n_jt - 1),
                        skip_group_check=(lo > 0 and jt != n_jt - 1),
                    )
            if jt_hi < n_jt:
                return (ots, vecsums), filler
            # denominator: all-ones matmuls -> column sums on all partitions;
            # one psum tile + one fused reciprocal for both heads
            den = mix_tile()
            for h in range(H_LOC):
                nc.tensor.matmul(den[:, h, :W], lhsT=ones,
                                 rhs=vecsums[0][:, h, :W],
                                 start=True, stop=True)
            recipb = rtmp.tile([128, H_LOC, TCH], f32, tag="recipb",
                               name="recipb")
            nc.vector.reciprocal(out=recipb[:, :, :W], in_=den[:, :, :W])
            for h in range(H_LOC):
                nc.vector.tensor_mul(out=yc[:, h, off:off + W],
                                     in0=ots[h][:, :W], in1=recipb[:, h, :W])
            return filler

        def cproj_quanta(q0, W, yc, off, pools=None, dve_only=False):
            """Partial c_proj for rows [q0, q0+W) as a list of work quanta.

            Each quantum emits half a 128-row tile (4 matmuls + one [128,
            1024] PSUM evacuation + its output DMA); the caller threads them
            into an attention span's jt loop so PE fills exp-gated gaps.
            pools: optional psum pool rotation (tail c_projs run when the
            proj/attention pools are idle -- deeper pipelining)."""
            obs = {}

            def quantum(tt, half, pool):
                gt = q0 // 128 + tt
                if half == 0:
                    obs[tt] = opool.tile([128, D], f16, tag="ob", name="ob")
                if pool is None:
                    ps = mix_tile()
                else:
                    ps = pool.tile([128, H_LOC, TCH], f32, tag="cp", name="cp")
                for nk in range(2):
                    nck = half * 2 + nk
                    for h in range(H_LOC):
                        nc.tensor.matmul(
                            ps[:, nk, :],
                            lhsT=yc[:, h,
                                    off + tt * 128:off + (tt + 1) * 128],
                            rhs=w_o[:, h, nck * 512:(nck + 1) * 512],
                            start=(h == 0),
                            stop=(h == H_LOC - 1),
                        )
                # evacuate [128, 1024] in one instr; alternate ACT/DVE;
                # one full-row DMA per gt (HWDGE descriptor-gen is a serial
                # 625ns/DMA resource worth conserving)
                osl = obs[tt][:, half * 1024:(half + 1) * 1024]
                if half == 0:
                    nc.scalar.copy(out=osl, in_=ps)
                else:
                    nc.vector.tensor_copy(out=osl, in_=ps)
                    nc.sync.dma_start(
                        out_d[gt * 128:(gt + 1) * 128, :], obs[tt][:])

            def quantum_nck(tt, nck, pool):
                """Pool-rotation variant: one nck per quantum, [128, 512]
                psum tiles from the (tail-idle) proj/attention rings."""
                gt = q0 // 128 + tt
                if nck == 0:
                    obs[tt] = opool.tile([128, D], f16, tag="ob", name="ob")
                ps = ps_tile(pool)
                for h in range(H_LOC):
                    nc.tensor.matmul(
                        ps,
                        lhsT=yc[:, h, off + tt * 128:off + (tt + 1) * 128],
                        rhs=w_o[:, h, nck * 512:(nck + 1) * 512],
                        start=(h == 0),
                        stop=(h == H_LOC - 1),
                    )
                osl = obs[tt][:, nck * 512:(nck + 1) * 512]
                if nck % 2 == 0 and not dve_only:
                    nc.scalar.copy(out=osl, in_=ps)
                else:
                    nc.vector.tensor_copy(out=osl, in_=ps)
                if nck == 3:
                    nc.sync.dma_start(
                        out_d[gt * 128:(gt + 1) * 128, :], obs[tt][:])

            if pools:
                return [
                    (lambda tt=tt, nck=nck,
                     pool=pools[(4 * tt + nck) % len(pools)]:
                     quantum_nck(tt, nck, pool))
                    for tt in range(W // 128) for nck in range(4)
                ]
            return [
                (lambda tt=tt, half=half: quantum(tt, half, None))
                for tt in range(W // 128) for half in range(2)
            ]

        def drain(filler):
            filler = iter(filler) if not hasattr(filler, "__next__") else filler
            for q in filler:
                q()

        # Emission order: projections stream in chunk order; each attention
        # chunk is emitted as soon as its projections exist. The last chunk
        # splits q from k/v so its early j-tiles overlap the projections, and
        # chunk 0's (tiny) attention is saved for the very end so the serial
        # tail after the final projections is as short as possible.
        xc0 = issue_x(0)
        qc0 = proj_q(0, xc0, qtag="qc0")
        drain(kv_quanta(0, xc0))
        yc0 = ypool.tile([128, H_LOC, TCH], f16, tag="yc0", name="yc0")
        xc1 = issue_x(1)
        qc1 = proj_q(1, xc1)
        drain(kv_quanta(1, xc1))
        yc1 = ypool.tile([128, H_LOC, TCH], f16, tag="yc", name="yc")
        xc2 = issue_x(2)
        # chunk c+1's k/v projections interleave into chunk c's attention:
        # they are PE-dense but ACT/DVE-light, exactly what the exp-gated
        # jt loop can absorb
        attn_span(TCH, TCH, qc1, 0, yc1, filler=kv_quanta(2, xc2))
        qc2 = proj_q(2, xc2)
        drain(cproj_quanta(TCH, TCH, yc1, 0))
        yc2 = ypool.tile([128, H_LOC, TCH], f16, tag="yc", name="yc")
        xc3 = issue_x(3)
        attn_span(2 * TCH, TCH, qc2, 0, yc2, filler=kv_quanta(3, xc3))
        qc3 = proj_q(3, xc3)
        drain(cproj_quanta(2 * TCH, TCH, yc2, 0))
        yc3 = ypool.tile([128, H_LOC, TCH], f16, tag="yc", name="yc")
        # chunk 0's (tiny) attention runs in the pre-attn3 window, its PV
        # accumulators in the now-idle proj psum ring; its c_proj then fills
        # attn3's exp-gated jt loop with DVE-only evacuations (ACT is
        # exp-saturated there)
        attn_span(0, TCH, qc0, 0, yc0, ot_pool=psum_p)
        q00 = cproj_quanta(0, TCH, yc0, 0, pools=(psum_p,), dve_only=True)
        rest = attn_span(3 * TCH, TCH, qc3, 0, yc3, filler=q00)
        drain(rest)
        drain(cproj_quanta(3 * TCH, TCH, yc3, 0, pools=(psum_ot, psum_p)))

    nc.compile()
    _CACHE["nc"] = nc
    return nc


def host_inputs(x, Wq, Wk, Wv, Wo):
    """Per-core input dicts (host-side shard + transpose + quantize + tables)."""
    import ml_dtypes

    f8 = ml_dtypes.float8_e4m3
    bf16 = ml_dtypes.bfloat16

    def pack_x8(a):  # (D, T) f32 -> (128, KO2, 2, T) fp8 DoubleRow layout
        return np.ascontiguousarray(
            a.reshape(KO2, 2, 128, T).transpose(2, 0, 1, 3)).astype(f8)

    x2 = np.ascontiguousarray(x.reshape(T, D).T).astype(np.float32)  # (D, T)
    x8 = pack_x8(x2)
    # fp8 residual (scaled 8x) for the v projection's second pass
    x8lo = pack_x8(
        8.0 * (x2 - x8.transpose(1, 2, 0, 3).reshape(D, T).astype(np.float32)))

    half = DH // 2  # 64
    af = (1.0 / 1024.0) ** np.linspace(0.0, 1.0, DH // 4, dtype=np.float32)
    af = np.concatenate([af, np.zeros(DH // 4, np.float32)])         # (64,)
    theta = np.arange(T, dtype=np.float32)[:, None] * af[None, :]    # (T, 64)
    cos = np.cos(theta).T.astype(np.float32)                         # (64, T)
    sin = np.sin(theta).T.astype(np.float32)
    ctab1 = np.concatenate([cos, cos], axis=0)                       # (128, T)
    stab1 = np.concatenate([sin, -sin], axis=0)
    # duplicated per head for head-fused rope ops: (128, H_LOC, T)
    ctab = np.repeat(ctab1[:, None, :], H_LOC, axis=1).astype(np.float16)
    stab = np.repeat(stab1[:, None, :], H_LOC, axis=1).astype(np.float32)

    roll = np.zeros((128, 128), np.float16)
    for p in range(128):
        roll[p, (p + half) % 128] = 1.0
    ones = np.ones((128, 128), np.float16)
    tri1 = np.triu(np.ones((128, 128), np.float16))  # tri[j, i] = i >= j
    tri = np.repeat(tri1[:, None, :], H_LOC, axis=1)

    shared = {
        "x8": x8, "x8lo": x8lo, "ctab": ctab, "stab": stab,
        "roll": roll, "ones": ones, "tri": tri,
    }

    def pack_pre(wt):  # pre-scaled (D, HD_LOC) f32 -> DoubleRow fp8 layout
        return np.ascontiguousarray(
            wt.reshape(KO2, 2, 128, HD_LOC).transpose(2, 0, 1, 3)).astype(f8)

    def pack_w8(w):  # (HD_LOC, D) slice -> (128, KO2, 2, HD_LOC) fp8, x WS
        return pack_pre((w.T * WS).astype(np.float32))

    in_maps = []
    for c in range(N_CORES):
        sl = slice(c * HD_LOC, (c + 1) * HD_LOC)
        wv_t = Wv[sl, :].T.astype(np.float32)               # (D, HD_LOC)
        wv8a = pack_pre(wv_t * WS)
        # residual of the 64x-quantized Wv, itself scaled 64x
        wv_res = wv_t - wv8a.transpose(1, 2, 0, 3).reshape(D, HD_LOC).astype(
            np.float32) / WS
        in_maps.append({
            **shared,
            "wq8": pack_w8(Wq[sl, :]),
            "wk8": pack_w8(Wk[sl, :]),
            "wv8a": wv8a,
            "wv8b": pack_pre(wv_t * 8.0),
            "wv8c": pack_pre(wv_res * WS),
            "wo16": np.ascontiguousarray((Wo[:, sl] / 3.0).T).astype(np.float16),
        })
    return in_maps


def _get_runner():
    """Build the program + a persistent jitted SPMD executable (once)."""
    if "runner" in _CACHE:
        return _CACHE["runner"]

    import jax
    import concourse.mybir as mybir
    from concourse.bass2jax import (
        _bass_exec_p,
        install_neuronx_cc_hook,
        partition_id_tensor,
    )
    from jax.experimental.shard_map import shard_map
    from jax.sharding import Mesh, PartitionSpec

    nc = build_program()
    install_neuronx_cc_hook()
    assert nc.dbg_addr is None
    pid_name = nc.partition_id_tensor.name if nc.partition_id_tensor else None

    in_names, out_names, out_avals, zero_outs = [], [], [], []
    for alloc in nc.m.functions[0].allocations:
        if not isinstance(alloc, mybir.MemoryLocationSet):
            continue
        name = alloc.memorylocations[0].name
        if alloc.kind == "ExternalInput":
            if name != pid_name:
                in_names.append(name)
        elif alloc.kind == "ExternalOutput":
            out_names.append(name)
            shape = tuple(alloc.tensor_shape)
            dtype = mybir.dt.np(alloc.dtype)
            out_avals.append(jax.core.ShapedArray(shape, dtype))
            zero_outs.append(np.zeros(shape, dtype))
    n_params = len(in_names)
    all_names = list(in_names) + list(out_names)
    if pid_name is not None:
        all_names.append(pid_name)
    donate = tuple(range(n_params, n_params + len(out_names)))

    def _body(*args):
        operands = list(args)
        if pid_name is not None:
            operands.append(partition_id_tensor())
        outs = _bass_exec_p.bind(
            *operands,
            out_avals=tuple(out_avals),
            in_names=tuple(all_names),
            out_names=tuple(out_names),
            lowering_input_output_aliases=(),
            sim_require_finite=True,
            sim_require_nnan=True,
            nc=nc,
        )
        return tuple(outs)

    devices = jax.devices()[:N_CORES]
    mesh = Mesh(np.asarray(devices), ("core",))
    in_specs = (PartitionSpec("core"),) * (n_params + len(out_names))
    out_specs = (PartitionSpec("core"),) * len(out_names)
    fn = jax.jit(
        shard_map(_body, mesh=mesh, in_specs=in_specs, out_specs=out_specs,
                  check_rep=False),
        donate_argnums=donate,
        keep_unused=True,
    )
    runner = (fn, in_names, out_names, out_avals, zero_outs)
    _CACHE["runner"] = runner
    return runner


def run_spmd(in_maps):
    """Execute the SPMD program; returns per-core output dicts."""
    fn, in_names, out_names, out_avals, zero_outs = _get_runner()
    concat_in = [
        np.concatenate([np.asarray(in_maps[c][n]) for c in range(N_CORES)], axis=0)
        for n in in_names
    ]
    concat_zeros = [
        np.zeros((N_CORES * z.shape[0], *z.shape[1:]), z.dtype) for z in zero_outs
    ]
    out_arrs = fn(*concat_in, *concat_zeros)
    return [
        {n: np.asarray(out_arrs[i]).reshape(N_CORES, *out_avals[i].shape)[c]
         for i, n in enumerate(out_names)}
        for c in range(N_CORES)
    ]


def kernel(x, Wq, Wk, Wv, Wo):
    in_maps = host_inputs(np.asarray(x), np.asarray(Wq), np.asarray(Wk),
                          np.asarray(Wv), np.asarray(Wo))
    results = run_spmd(in_maps)
    out = results[0]["outp"].astype(np.float64)
    for c in range(1, N_CORES):
        out += results[c]["outp"].astype(np.float64)
    return out.astype(np.float32).reshape(1, T, D)


# revision 56
# speedup vs baseline: 1.4008x; 1.0113x over previous
"""Causal self-attention with RoPE, tensor-parallel over heads on 8 TRN2 NeuronCores.

Model (from the reference):
    q/k/v = x @ W{q,k,v}.T          x: (1, 2048, 2048), 16 heads x 128 head_dim
    rope(q), rope(k)                half-rotation, 32 nonzero freqs
    causal softmax(q k^T / sqrt(128)) @ v
    out = (y / 3) @ Wo.T

Sharding: 2 heads per core. Each core computes its heads' q/k/v projections,
attention, and a partial c_proj (its 256 columns of the hd contraction);
the host sums the 8 partial outputs (the "all-reduce after c_proj").

Per-core kernel layout choices:
  - Everything transposed so the contraction dim is always on partitions:
    host supplies x in fp8/bf16 k-subtile layouts plus pre-transposed,
    pre-quantized weight slices.
  - q/k projections in fp8e4 DoubleRow perf mode (two 128-row k-subtiles
    contracted per pass): weights pre-scaled by 64 on host (their native
    magnitude ~0.02 is subnormal in e4m3); the 64*64 factor is folded into
    the softmax exp scale.
  - v projection in bf16; everything downstream of the projections (roped
    q/k, v, attention probabilities, y, Wo) lives in fp16: DVE runs 2x on
    16-bit operands and fp16's 10-bit mantissa keeps errors ~5e-4.
  - Scores computed transposed (S^T[j, i] blocks) so the P @ V matmul needs
    no transposes.
  - Softmax without max-subtraction (scores are provably tiny: |s| < ~2),
    denominator via DVE/GPSIMD accumulation + all-ones matmuls.
  - RoPE via a PE roll matmul + 3 DVE ops, both heads fused per op.
  - Output partials in fp16 (halves the output DMA; host sums in float64).
"""

import numpy as np

T = 2048
D = 2048
H = 16
DH = 128
N_CORES = 8
H_LOC = H // N_CORES          # heads per core = 2
HD_LOC = H_LOC * DH           # local head dims = 256
TCH = 512                     # query-chunk width
N_CH = T // TCH               # 4 chunks
KO = D // 128                 # 16 contraction subtiles
KO2 = KO // 2                 # 8 DoubleRow k-subtile pairs
WS = 64.0                     # host prescale on Wq/Wk before fp8 quantization
SCALE = (DH ** 0.5) / DH      # 1/sqrt(128)

_CACHE = {}


def build_program():
    """Build (once) the single-core Bass program shared by all 8 cores."""
    if "nc" in _CACHE:
        return _CACHE["nc"]

    from contextlib import ExitStack

    import concourse.bacc as bacc
    import concourse.mybir as mybir
    import concourse.tile as tile

    f32 = mybir.dt.float32
    bf16 = mybir.dt.bfloat16
    f16 = mybir.dt.float16
    f8 = mybir.dt.float8e4
    EXP = mybir.ActivationFunctionType.Exp
    DR = mybir.MatmulPerfMode.DoubleRow

    nc = bacc.Bacc("TRN2", target_bir_lowering=False)

    x8_d = nc.dram_tensor("x8", (128, KO2, 2, T), f8, kind="ExternalInput")
    xl_d = nc.dram_tensor("x8lo", (128, KO2, 2, T), f8, kind="ExternalInput")
    wq_d = nc.dram_tensor("wq8", (128, KO2, 2, HD_LOC), f8, kind="ExternalInput")
    wk_d = nc.dram_tensor("wk8", (128, KO2, 2, HD_LOC), f8, kind="ExternalInput")
    wva_d = nc.dram_tensor("wv8a", (128, KO2, 2, HD_LOC), f8, kind="ExternalInput")
    wvb_d = nc.dram_tensor("wv8b", (128, KO2, 2, HD_LOC), f8, kind="ExternalInput")
    wvc_d = nc.dram_tensor("wv8c", (128, KO2, 2, HD_LOC), f8, kind="ExternalInput")
    wo_d = nc.dram_tensor("wo16", (HD_LOC, D), f16, kind="ExternalInput")
    ct_d = nc.dram_tensor("ctab", (128, H_LOC, T), f16, kind="ExternalInput")
    st_d = nc.dram_tensor("stab", (128, H_LOC, T), f16, kind="ExternalInput")
    roll_d = nc.dram_tensor("roll", (128, 128), f16, kind="ExternalInput")
    ones_d = nc.dram_tensor("ones", (128, 128), f16, kind="ExternalInput")
    tri_d = nc.dram_tensor("tri", (128, H_LOC, 128), f16, kind="ExternalInput")
    out_d = nc.dram_tensor("outp", (T, D), f16, kind="ExternalOutput")

    wo_r = wo_d[:].rearrange("(h p) d -> p h d", p=128)

    with tile.TileContext(nc) as tc, ExitStack() as ctx:
        persist = ctx.enter_context(tc.tile_pool(name="persist", bufs=1))
        qpool = ctx.enter_context(tc.tile_pool(name="qpool", bufs=2))
        ypool = ctx.enter_context(tc.tile_pool(name="ypool", bufs=2))
        xpool = ctx.enter_context(tc.tile_pool(name="xpool", bufs=2))
        ptpool = ctx.enter_context(tc.tile_pool(name="ptpool", bufs=4))
        rtmp = ctx.enter_context(tc.tile_pool(name="rtmp", bufs=1))
        spool = ctx.enter_context(tc.tile_pool(name="spool", bufs=2))
        opool = ctx.enter_context(tc.tile_pool(name="opool", bufs=6))
        psum_p = ctx.enter_context(tc.tile_pool(name="psum_p", bufs=2, space="PSUM"))
        psum_mix = ctx.enter_context(tc.tile_pool(name="psum_mix", bufs=2, space="PSUM"))
        psum_ot = ctx.enter_context(tc.tile_pool(name="psum_ot", bufs=2, space="PSUM"))

        def ps_tile(pool=None):
            return (pool or psum_p).tile([128, TCH], f32, tag="ps", name="ps")

        def mix_tile():
            return psum_mix.tile([128, H_LOC, TCH], f32, tag="mix", name="mix")

        # --- resident tensors ---
        w_q = persist.tile([128, KO2, 2, HD_LOC], f8, tag="w_q")
        w_k = persist.tile([128, KO2, 2, HD_LOC], f8, tag="w_k")
        w_va = persist.tile([128, KO2, 2, HD_LOC], f8, tag="w_va")
        w_vb = persist.tile([128, KO2, 2, HD_LOC], f8, tag="w_vb")
        w_vc = persist.tile([128, KO2, 2, HD_LOC], f8, tag="w_vc")
        w_o = persist.tile([128, H_LOC, D], f16, tag="w_o")
        kt = persist.tile([128, H_LOC, T], f16, tag="kt")
        vt = persist.tile([128, KO, HD_LOC], f16, tag="vt")
        ctab = persist.tile([128, H_LOC, T], f16, tag="ctab")
        stab = persist.tile([128, H_LOC, T], f16, tag="stab")
        roll = persist.tile([128, 128], f16, tag="roll")
        ones = persist.tile([128, 128], f16, tag="ones")
        tri = persist.tile([128, H_LOC, 128], f16, tag="tri")

        def issue_x(c):
            """Queue the x chunk DMAs for chunk c (weights too on chunk 0)."""
            cs = c * TCH
            x8c = xpool.tile([128, KO2, 2, TCH], f8, tag="x8c", name="x8c")
            xloc = xpool.tile([128, KO2, 2, TCH], f8, tag="xloc", name="xloc")
            if c == 0:
                # ordered so the first PE work (q-proj, rope) unblocks
                # soonest; first transfers split+interleaved so the opening
                # matmul group can start after half the data has landed
                nc.sync.dma_start(x8c[:, :4], x8_d[:, :4, :, cs:cs + TCH])
                nc.sync.dma_start(w_q[:, :4], wq_d[:, :4])
                nc.sync.dma_start(x8c[:, 4:], x8_d[:, 4:, :, cs:cs + TCH])
                nc.sync.dma_start(w_q[:, 4:], wq_d[:, 4:])
                nc.sync.dma_start(w_k[:], wk_d[:])
                nc.sync.dma_start(roll[:], roll_d[:])
                nc.sync.dma_start(ctab[:], ct_d[:])
                nc.sync.dma_start(stab[:], st_d[:])
                nc.sync.dma_start(w_va[:], wva_d[:])
                nc.sync.dma_start(xloc[:], xl_d[:, :, :, cs:cs + TCH])
                nc.sync.dma_start(w_vb[:], wvb_d[:])
                nc.sync.dma_start(w_vc[:], wvc_d[:])
                nc.sync.dma_start(tri[:], tri_d[:])
                nc.sync.dma_start(ones[:], ones_d[:])
                nc.sync.dma_start(w_o[:], wo_r)
            else:
                nc.sync.dma_start(x8c[:], x8_d[:, :, :, cs:cs + TCH])
                nc.sync.dma_start(xloc[:], xl_d[:, :, :, cs:cs + TCH])
            return (x8c, xloc)

        def rope(sl, cs):
            """RoPE in place: y = x*C + roll64(x)*S', heads fused per DVE op."""
            rolled = mix_tile()
            for h in range(H_LOC):
                nc.tensor.matmul(rolled[:, h, :], lhsT=roll, rhs=sl[:, h, :],
                                 start=True, stop=True)
            a = rtmp.tile([128, H_LOC, TCH], f16, tag="ra", name="ra")
            b = rtmp.tile([128, H_LOC, TCH], f16, tag="rb", name="rb")
            nc.vector.tensor_mul(out=a, in0=sl, in1=ctab[:, :, cs:cs + TCH])
            nc.vector.tensor_mul(out=b, in0=rolled, in1=stab[:, :, cs:cs + TCH])
            nc.vector.tensor_add(out=sl, in0=a, in1=b)

        def qk_head(w_sb, x8c, dsl, h):
            """One head's q-or-k projection: 16 DoubleRow matmuls + copy."""
            ps = ps_tile()
            for tp in range(2):
                for jko in range(KO2):
                    nc.tensor.matmul(
                        ps[:, tp * 256:(tp + 1) * 256],
                        lhsT=w_sb[:, jko, :, h * 128:(h + 1) * 128],
                        rhs=x8c[:, jko, :, tp * 256:(tp + 1) * 256],
                        start=(jko == 0),
                        stop=(jko == KO2 - 1),
                        perf_mode=DR,
                    )
            nc.scalar.copy(out=dsl, in_=ps)

        def v_half(c, xc, tu):
            """Half a chunk's v projection: 3-pass split-fp8 DoubleRow.

            v = x8*Wva(64w) + xlo(8dx)*Wvb(8w) + x8*Wvc(64dw); PSUM holds
            64*v, the evacuation copy scales by 1/64."""
            x8c, xloc = xc
            passes = ((x8c, w_va), (xloc, w_vb), (x8c, w_vc))
            ps = ps_tile()
            for tt in (2 * tu, 2 * tu + 1):
                sub = ps[:, (tt % 2) * HD_LOC:(tt % 2 + 1) * HD_LOC]
                for pi, (xt, wt) in enumerate(passes):
                    for jko in range(KO2):
                        nc.tensor.matmul(
                            sub,
                            lhsT=xt[:, jko, :, tt * 128:(tt + 1) * 128],
                            rhs=wt[:, jko, :, :],
                            start=(pi == 0 and jko == 0),
                            stop=(pi == 2 and jko == KO2 - 1),
                            perf_mode=DR,
                        )
            gt2 = c * 2 + tu
            nc.scalar.mul(out=vt[:, 2 * gt2:2 * gt2 + 2, :], in_=ps,
                          mul=1.0 / WS)

        def proj_q(c, xc, qtag="qc"):
            """q projection + its rope for t-chunk c."""
            qc = qpool.tile([128, H_LOC, TCH], f16, tag=qtag, name="qc")
            for h in range(H_LOC):
                qk_head(w_q, xc[0], qc[:, h, :], h)
            rope(qc[:, :, :], c * TCH)
            return qc

        def kv_quanta(c, xc):
            """k/v projections for chunk c as quanta (PE-heavy, ACT-light) --
            interleaved into the previous chunk's attention span."""
            cs = c * TCH

            def k_head(h):
                qk_head(w_k, xc[0], kt[:, h, cs:cs + TCH], h)

            return [
                lambda: k_head(0),
                lambda: k_head(1),
                lambda: rope(kt[:, :, cs:cs + TCH], cs),
                lambda: v_half(c, xc, 0),
                lambda: v_half(c, xc, 1),
            ]

        def attn_span(q0, W, qc, off, yc, jt_lo=0, jt_hi=None,
                      state=None, ot_pool=None, filler=(), fill_per_jt=1):
            """Causal attention for queries [q0, q0+W), heads interleaved.

            q0 must be 128-aligned; W in {256, 512}. qc holds the chunk's
            roped queries; off is q0's offset within qc/yc."""
            d0 = q0 // 128          # first diagonal j-tile
            n_jt = d0 + W // 128
            if state is None:
                ots = [ps_tile(ot_pool or psum_ot) for _ in range(H_LOC)]
                vecsums = [spool.tile([128, H_LOC, TCH], f16,
                                      tag=f"vecsum{par}", name="vecsum")
                           for par in range(2)]
            else:
                ots, vecsums = state
            if jt_hi is None:
                jt_hi = n_jt
            filler = iter(filler) if not hasattr(filler, "__next__") else filler
            for jt in range(jt_lo, jt_hi):
                # interleave deferred work (previous chunk's c_proj) into the
                # jt loop: PE's stream is in-order per engine, so this is the
                # only way it can fill the exp-gated gaps between j-tiles
                for _ in range(fill_per_jt):
                    q = next(filler, None)
                    if q is not None:
                        q()
                pair = mix_tile()
                m = jt - d0
                # diagonal block: cols < 128m fully masked -- never written,
                # never read (partial-width ops)
                lo = 128 * m if m > 0 else 0
                # score matmul skips dead columns too, but only while the
                # moving dim stays >= 256 (full rate)
                slo = lo if W - lo >= 256 else 0
                for h in range(H_LOC):
                    nc.tensor.matmul(
                        pair[:, h, slo:W],
                        lhsT=kt[:, h, jt * 128:(jt + 1) * 128],
                        rhs=qc[:, h, off + slo:off + W],
                        start=True,
                        stop=True,
                    )
                pt = ptpool.tile([128, H_LOC, TCH], f16, tag="pt", name="pt")
                # both heads in ONE activation call (strided AP when lo > 0);
                # q/k carry the 64x host prescale each -> 1/4096 here
                nc.scalar.activation(out=pt[:, :, lo:W], in_=pair[:, :, lo:W],
                                     func=EXP, scale=SCALE / (WS * WS))
                if m >= 0:
                    # mask the diagonal block, both heads in one op
                    nc.vector.tensor_mul(
                        out=pt[:, :, 128 * m:128 * (m + 1)],
                        in0=pt[:, :, 128 * m:128 * (m + 1)],
                        in1=tri[:],
                    )
                # probability row-sum accumulator (all DVE: f16 runs 2x and
                # GPSIMD's 0.42-efficiency adds would chain on the critical
                # path). jt==0 initializes via copy; on q0=0 spans jt==1 is
                # diagonal with cols < 128 unwritten, so never full-copy there.
                vs = vecsums[0]
                if jt == 0:
                    nc.vector.tensor_copy(out=vs[:, :, :W], in_=pt[:, :, :W])
                else:
                    nc.vector.tensor_add(out=vs[:, :, lo:W], in0=vs[:, :, lo:W],
                                         in1=pt[:, :, lo:W])
                for h in range(H_LOC):
                    # partial-width diagonal writes skip the (bank-granular)
                    # psum group check -- EXCEPT the last j-tile, whose
                    # stop must be bookkept so the ymul read sees a closed
                    # group
                    nc.tensor.matmul(
                        ots[h][:, lo:W],
                        lhsT=vt[:, jt, h * 128:(h + 1) * 128],
                        rhs=pt[:, h, lo:W],
                        start=(jt == 0),
                        stop=(jt == n_jt - 1),
                        skip_group_check=(lo > 0 and jt != n_jt - 1),
                    )
            if jt_hi < n_jt:
                return (ots, vecsums), filler
            # denominator: all-ones matmuls -> column sums on all partitions;
            # one psum tile + one fused reciprocal for both heads
            den = mix_tile()
            for h in range(H_LOC):
                nc.tensor.matmul(den[:, h, :W], lhsT=ones,
                                 rhs=vecsums[0][:, h, :W],
                                 start=True, stop=True)
            recipb = rtmp.tile([128, H_LOC, TCH], f32, tag="recipb",
                               name="recipb")
            nc.vector.reciprocal(out=recipb[:, :, :W], in_=den[:, :, :W])
            for h in range(H_LOC):
                nc.vector.tensor_mul(out=yc[:, h, off:off + W],
                                     in0=ots[h][:, :W], in1=recipb[:, h, :W])
            return filler

        def cproj_quanta(q0, W, yc, off, pools=None, dve_only=False):
            """Partial c_proj for rows [q0, q0+W) as a list of work quanta.

            Each quantum emits half a 128-row tile (4 matmuls + one [128,
            1024] PSUM evacuation + its output DMA); the caller threads them
            into an attention span's jt loop so PE fills exp-gated gaps.
            pools: optional psum pool rotation (tail c_projs run when the
            proj/attention pools are idle -- deeper pipelining)."""
            obs = {}

            def quantum(tt, half, pool):
                gt = q0 // 128 + tt
                if half == 0:
                    obs[tt] = opool.tile([128, D], f16, tag="ob", name="ob")
                if pool is None:
                    ps = mix_tile()
                else:
                    ps = pool.tile([128, H_LOC, TCH], f32, tag="cp", name="cp")
                for nk in range(2):
                    nck = half * 2 + nk
                    for h in range(H_LOC):
                        nc.tensor.matmul(
                            ps[:, nk, :],
                            lhsT=yc[:, h,
                                    off + tt * 128:off + (tt + 1) * 128],
                            rhs=w_o[:, h, nck * 512:(nck + 1) * 512],
                            start=(h == 0),
                            stop=(h == H_LOC - 1),
                        )
                # evacuate [128, 1024] in one instr; alternate ACT/DVE;
                # one full-row DMA per gt (HWDGE descriptor-gen is a serial
                # 625ns/DMA resource worth conserving)
                osl = obs[tt][:, half * 1024:(half + 1) * 1024]
                if half == 0:
                    nc.scalar.copy(out=osl, in_=ps)
                else:
                    nc.vector.tensor_copy(out=osl, in_=ps)
                    nc.sync.dma_start(
                        out_d[gt * 128:(gt + 1) * 128, :], obs[tt][:])

            def quantum_nck(tt, nck, pool):
                """Pool-rotation variant: one nck per quantum, [128, 512]
                psum tiles from the (tail-idle) proj/attention rings."""
                gt = q0 // 128 + tt
                if nck == 0:
                    obs[tt] = opool.tile([128, D], f16, tag="ob", name="ob")
                ps = ps_tile(pool)
                for h in range(H_LOC):
                    nc.tensor.matmul(
                        ps,
                        lhsT=yc[:, h, off + tt * 128:off + (tt + 1) * 128],
                        rhs=w_o[:, h, nck * 512:(nck + 1) * 512],
                        start=(h == 0),
                        stop=(h == H_LOC - 1),
                    )
                osl = obs[tt][:, nck * 512:(nck + 1) * 512]
                if nck % 2 == 0 and not dve_only:
                    nc.scalar.copy(out=osl, in_=ps)
                else:
                    nc.vector.tensor_copy(out=osl, in_=ps)
                if nck % 2 == 1:
                    # per-half DMAs: the tail has HWDGE to spare and the
                    # earlier transfer start shortens the final drain
                    nc.sync.dma_start(
                        out_d[gt * 128:(gt + 1) * 128,
                              (nck - 1) * 512:(nck + 1) * 512],
                        obs[tt][:, (nck - 1) * 512:(nck + 1) * 512])

            if pools:
                return [
                    (lambda tt=tt, nck=nck,
                     pool=pools[(4 * tt + nck) % len(pools)]:
                     quantum_nck(tt, nck, pool))
                    for tt in range(W // 128) for nck in range(4)
                ]
            return [
                (lambda tt=tt, half=half: quantum(tt, half, None))
                for tt in range(W // 128) for half in range(2)
            ]

        def drain(filler):
            filler = iter(filler) if not hasattr(filler, "__next__") else filler
            for q in filler:
                if q is not None:
                    q()

        # Emission order: projections stream in chunk order; each attention
        # chunk is emitted as soon as its projections exist. The last chunk
        # splits q from k/v so its early j-tiles overlap the projections, and
        # chunk 0's (tiny) attention is saved for the very end so the serial
        # tail after the final projections is as short as possible.
        xc0 = issue_x(0)
        qc0 = proj_q(0, xc0, qtag="qc0")
        drain(kv_quanta(0, xc0))
        yc0 = ypool.tile([128, H_LOC, TCH], f16, tag="yc0", name="yc0")
        xc1 = issue_x(1)
        qc1 = proj_q(1, xc1)
        drain(kv_quanta(1, xc1))
        yc1 = ypool.tile([128, H_LOC, TCH], f16, tag="yc", name="yc")
        xc2 = issue_x(2)
        # chunk c+1's k/v projections interleave into chunk c's attention:
        # they are PE-dense but ACT/DVE-light, exactly what the exp-gated
        # jt loop can absorb
        attn_span(TCH, TCH, qc1, 0, yc1, filler=kv_quanta(2, xc2))
        qc2 = proj_q(2, xc2)
        drain(cproj_quanta(TCH, TCH, yc1, 0))
        yc2 = ypool.tile([128, H_LOC, TCH], f16, tag="yc", name="yc")
        xc3 = issue_x(3)
        attn_span(2 * TCH, TCH, qc2, 0, yc2, filler=kv_quanta(3, xc3))
        qc3 = proj_q(3, xc3)
        drain(cproj_quanta(2 * TCH, TCH, yc2, 0))
        yc3 = ypool.tile([128, H_LOC, TCH], f16, tag="yc", name="yc")
        # chunk 0's (tiny) attention runs in the pre-attn3 window, its PV
        # accumulators in the now-idle proj psum ring; its c_proj then fills
        # attn3's exp-gated jt loop with DVE-only evacuations (ACT is
        # exp-saturated there)
        attn_span(0, TCH, qc0, 0, yc0, ot_pool=psum_p)
        q00 = cproj_quanta(0, TCH, yc0, 0, pools=(psum_p,), dve_only=True)
        # pull one evacuation every OTHER j-tile: DVE also carries the
        # vecsum chain there and saturates at 1/jt
        paced = (q for pair in zip(q00, [None] * len(q00)) for q in pair)
        rest = attn_span(3 * TCH, TCH, qc3, 0, yc3, filler=paced)
        drain(rest)
        drain(cproj_quanta(3 * TCH, TCH, yc3, 0, pools=(psum_ot, psum_p)))

    nc.compile()
    _CACHE["nc"] = nc
    return nc


def host_inputs(x, Wq, Wk, Wv, Wo):
    """Per-core input dicts (host-side shard + transpose + quantize + tables)."""
    import ml_dtypes

    f8 = ml_dtypes.float8_e4m3
    bf16 = ml_dtypes.bfloat16

    def pack_x8(a):  # (D, T) f32 -> (128, KO2, 2, T) fp8 DoubleRow layout
        return np.ascontiguousarray(
            a.reshape(KO2, 2, 128, T).transpose(2, 0, 1, 3)).astype(f8)

    x2 = np.ascontiguousarray(x.reshape(T, D).T).astype(np.float32)  # (D, T)
    x8 = pack_x8(x2)
    # fp8 residual (scaled 8x) for the v projection's second pass
    x8lo = pack_x8(
        8.0 * (x2 - x8.transpose(1, 2, 0, 3).reshape(D, T).astype(np.float32)))

    half = DH // 2  # 64
    af = (1.0 / 1024.0) ** np.linspace(0.0, 1.0, DH // 4, dtype=np.float32)
    af = np.concatenate([af, np.zeros(DH // 4, np.float32)])         # (64,)
    theta = np.arange(T, dtype=np.float32)[:, None] * af[None, :]    # (T, 64)
    cos = np.cos(theta).T.astype(np.float32)                         # (64, T)
    sin = np.sin(theta).T.astype(np.float32)
    ctab1 = np.concatenate([cos, cos], axis=0)                       # (128, T)
    stab1 = np.concatenate([sin, -sin], axis=0)
    # duplicated per head for head-fused rope ops: (128, H_LOC, T)
    ctab = np.repeat(ctab1[:, None, :], H_LOC, axis=1).astype(np.float16)
    stab = np.repeat(stab1[:, None, :], H_LOC, axis=1).astype(np.float32)

    roll = np.zeros((128, 128), np.float16)
    for p in range(128):
        roll[p, (p + half) % 128] = 1.0
    ones = np.ones((128, 128), np.float16)
    tri1 = np.triu(np.ones((128, 128), np.float16))  # tri[j, i] = i >= j
    tri = np.repeat(tri1[:, None, :], H_LOC, axis=1)

    shared = {
        "x8": x8, "x8lo": x8lo, "ctab": ctab, "stab": stab,
        "roll": roll, "ones": ones, "tri": tri,
    }

    def pack_pre(wt):  # pre-scaled (D, HD_LOC) f32 -> DoubleRow fp8 layout
        return np.ascontiguousarray(
            wt.reshape(KO2, 2, 128, HD_LOC).transpose(2, 0, 1, 3)).astype(f8)

    def pack_w8(w):  # (HD_LOC, D) slice -> (128, KO2, 2, HD_LOC) fp8, x WS
        return pack_pre((w.T * WS).astype(np.float32))

    in_maps = []
    for c in range(N_CORES):
        sl = slice(c * HD_LOC, (c + 1) * HD_LOC)
        wv_t = Wv[sl, :].T.astype(np.float32)               # (D, HD_LOC)
        wv8a = pack_pre(wv_t * WS)
        # residual of the 64x-quantized Wv, itself scaled 64x
        wv_res = wv_t - wv8a.transpose(1, 2, 0, 3).reshape(D, HD_LOC).astype(
            np.float32) / WS
        in_maps.append({
            **shared,
            "wq8": pack_w8(Wq[sl, :]),
            "wk8": pack_w8(Wk[sl, :]),
            "wv8a": wv8a,
            "wv8b": pack_pre(wv_t * 8.0),
            "wv8c": pack_pre(wv_res * WS),
            "wo16": np.ascontiguousarray((Wo[:, sl] / 3.0).T).astype(np.float16),
        })
    return in_maps


def _get_runner():
    """Build the program + a persistent jitted SPMD executable (once)."""
    if "runner" in _CACHE:
        return _CACHE["runner"]

    import jax
    import concourse.mybir as mybir
    from concourse.bass2jax import (
        _bass_exec_p,
        install_neuronx_cc_hook,
        partition_id_tensor,
    )
    from jax.experimental.shard_map import shard_map
    from jax.sharding import Mesh, PartitionSpec

    nc = build_program()
    install_neuronx_cc_hook()
    assert nc.dbg_addr is None
    pid_name = nc.partition_id_tensor.name if nc.partition_id_tensor else None

    in_names, out_names, out_avals, zero_outs = [], [], [], []
    for alloc in nc.m.functions[0].allocations:
        if not isinstance(alloc, mybir.MemoryLocationSet):
            continue
        name = alloc.memorylocations[0].name
        if alloc.kind == "ExternalInput":
            if name != pid_name:
                in_names.append(name)
        elif alloc.kind == "ExternalOutput":
            out_names.append(name)
            shape = tuple(alloc.tensor_shape)
            dtype = mybir.dt.np(alloc.dtype)
            out_avals.append(jax.core.ShapedArray(shape, dtype))
            zero_outs.append(np.zeros(shape, dtype))
    n_params = len(in_names)
    all_names = list(in_names) + list(out_names)
    if pid_name is not None:
        all_names.append(pid_name)
    donate = tuple(range(n_params, n_params + len(out_names)))

    def _body(*args):
        operands = list(args)
        if pid_name is not None:
            operands.append(partition_id_tensor())
        outs = _bass_exec_p.bind(
            *operands,
            out_avals=tuple(out_avals),
            in_names=tuple(all_names),
            out_names=tuple(out_names),
            lowering_input_output_aliases=(),
            sim_require_finite=True,
            sim_require_nnan=True,
            nc=nc,
        )
        return tuple(outs)

    devices = jax.devices()[:N_CORES]
    mesh = Mesh(np.asarray(devices), ("core",))
    in_specs = (PartitionSpec("core"),) * (n_params + len(out_names))
    out_specs = (PartitionSpec("core"),) * len(out_names)
    fn = jax.jit(
        shard_map(_body, mesh=mesh, in_specs=in_specs, out_specs=out_specs,
                  check_rep=False),
        donate_argnums=donate,
        keep_unused=True,
    )
    runner = (fn, in_names, out_names, out_avals, zero_outs)
    _CACHE["runner"] = runner
    return runner


def run_spmd(in_maps):
    """Execute the SPMD program; returns per-core output dicts."""
    fn, in_names, out_names, out_avals, zero_outs = _get_runner()
    concat_in = [
        np.concatenate([np.asarray(in_maps[c][n]) for c in range(N_CORES)], axis=0)
        for n in in_names
    ]
    concat_zeros = [
        np.zeros((N_CORES * z.shape[0], *z.shape[1:]), z.dtype) for z in zero_outs
    ]
    out_arrs = fn(*concat_in, *concat_zeros)
    return [
        {n: np.asarray(out_arrs[i]).reshape(N_CORES, *out_avals[i].shape)[c]
         for i, n in enumerate(out_names)}
        for c in range(N_CORES)
    ]


def kernel(x, Wq, Wk, Wv, Wo):
    in_maps = host_inputs(np.asarray(x), np.asarray(Wq), np.asarray(Wk),
                          np.asarray(Wv), np.asarray(Wo))
    results = run_spmd(in_maps)
    out = results[0]["outp"].astype(np.float64)
    for c in range(1, N_CORES):
        out += results[c]["outp"].astype(np.float64)
    return out.astype(np.float32).reshape(1, T, D)


# revision 57
# speedup vs baseline: 1.4973x; 1.0689x over previous
"""Causal self-attention with RoPE, tensor-parallel over heads on 8 TRN2 NeuronCores.

Model (from the reference):
    q/k/v = x @ W{q,k,v}.T          x: (1, 2048, 2048), 16 heads x 128 head_dim
    rope(q), rope(k)                half-rotation, 32 nonzero freqs
    causal softmax(q k^T / sqrt(128)) @ v
    out = (y / 3) @ Wo.T

Sharding: 2 heads per core. Each core computes its heads' q/k/v projections,
attention, and a partial c_proj (its 256 columns of the hd contraction);
the host sums the 8 partial outputs (the "all-reduce after c_proj").

Per-core kernel layout choices:
  - Everything transposed so the contraction dim is always on partitions:
    host supplies x in fp8/bf16 k-subtile layouts plus pre-transposed,
    pre-quantized weight slices.
  - q/k projections in fp8e4 DoubleRow perf mode (two 128-row k-subtiles
    contracted per pass): weights pre-scaled by 64 on host (their native
    magnitude ~0.02 is subnormal in e4m3); the 64*64 factor is folded into
    the softmax exp scale.
  - v projection in bf16; everything downstream of the projections (roped
    q/k, v, attention probabilities, y, Wo) lives in fp16: DVE runs 2x on
    16-bit operands and fp16's 10-bit mantissa keeps errors ~5e-4.
  - Scores computed transposed (S^T[j, i] blocks) so the P @ V matmul needs
    no transposes.
  - Softmax without max-subtraction (scores are provably tiny: |s| < ~2),
    denominator via DVE/GPSIMD accumulation + all-ones matmuls.
  - RoPE via a PE roll matmul + 3 DVE ops, both heads fused per op.
  - Output partials in fp16 (halves the output DMA; host sums in float64).
"""

import numpy as np

T = 2048
D = 2048
H = 16
DH = 128
N_CORES = 8
H_LOC = H // N_CORES          # heads per core = 2
HD_LOC = H_LOC * DH           # local head dims = 256
TCH = 512                     # query-chunk width
N_CH = T // TCH               # 4 chunks
KO = D // 128                 # 16 contraction subtiles
KO2 = KO // 2                 # 8 DoubleRow k-subtile pairs
WS = 64.0                     # host prescale on Wq/Wk before fp8 quantization
SCALE = (DH ** 0.5) / DH      # 1/sqrt(128)

_CACHE = {}


def build_program():
    """Build (once) the single-core Bass program shared by all 8 cores."""
    if "nc" in _CACHE:
        return _CACHE["nc"]

    from contextlib import ExitStack

    import concourse.bacc as bacc
    import concourse.mybir as mybir
    import concourse.tile as tile

    f32 = mybir.dt.float32
    bf16 = mybir.dt.bfloat16
    f16 = mybir.dt.float16
    f8 = mybir.dt.float8e4
    EXP = mybir.ActivationFunctionType.Exp
    DR = mybir.MatmulPerfMode.DoubleRow

    nc = bacc.Bacc("TRN2", target_bir_lowering=False)

    x8_d = nc.dram_tensor("x8", (128, KO2, 2, T), f8, kind="ExternalInput")
    xl_d = nc.dram_tensor("x8lo", (128, KO2, 2, T), f8, kind="ExternalInput")
    wq_d = nc.dram_tensor("wq8", (128, KO2, 2, HD_LOC), f8, kind="ExternalInput")
    wk_d = nc.dram_tensor("wk8", (128, KO2, 2, HD_LOC), f8, kind="ExternalInput")
    wva_d = nc.dram_tensor("wv8a", (128, KO2, 2, HD_LOC), f8, kind="ExternalInput")
    wvb_d = nc.dram_tensor("wv8b", (128, KO2, 2, HD_LOC), f8, kind="ExternalInput")
    wvc_d = nc.dram_tensor("wv8c", (128, KO2, 2, HD_LOC), f8, kind="ExternalInput")
    wo_d = nc.dram_tensor("wo16", (HD_LOC, D), f16, kind="ExternalInput")
    ct_d = nc.dram_tensor("ctab", (128, H_LOC, T), f16, kind="ExternalInput")
    st_d = nc.dram_tensor("stab", (128, H_LOC, T), f16, kind="ExternalInput")
    roll_d = nc.dram_tensor("roll", (128, 128), f16, kind="ExternalInput")
    ones_d = nc.dram_tensor("ones", (128, 128), f16, kind="ExternalInput")
    tri_d = nc.dram_tensor("tri", (128, H_LOC, 128), f16, kind="ExternalInput")
    out_d = nc.dram_tensor("outp", (T, D), f16, kind="ExternalOutput")

    wo_r = wo_d[:].rearrange("(h p) d -> p h d", p=128)

    with tile.TileContext(nc) as tc, ExitStack() as ctx:
        persist = ctx.enter_context(tc.tile_pool(name="persist", bufs=1))
        qpool = ctx.enter_context(tc.tile_pool(name="qpool", bufs=2))
        ypool = ctx.enter_context(tc.tile_pool(name="ypool", bufs=2))
        xpool = ctx.enter_context(tc.tile_pool(name="xpool", bufs=2))
        ptpool = ctx.enter_context(tc.tile_pool(name="ptpool", bufs=4))
        rtmp = ctx.enter_context(tc.tile_pool(name="rtmp", bufs=1))
        spool = ctx.enter_context(tc.tile_pool(name="spool", bufs=2))
        opool = ctx.enter_context(tc.tile_pool(name="opool", bufs=6))
        psum_p = ctx.enter_context(tc.tile_pool(name="psum_p", bufs=2, space="PSUM"))
        psum_mix = ctx.enter_context(tc.tile_pool(name="psum_mix", bufs=2, space="PSUM"))
        psum_ot = ctx.enter_context(tc.tile_pool(name="psum_ot", bufs=2, space="PSUM"))

        def ps_tile(pool=None):
            return (pool or psum_p).tile([128, TCH], f32, tag="ps", name="ps")

        def mix_tile():
            return psum_mix.tile([128, H_LOC, TCH], f32, tag="mix", name="mix")

        # --- resident tensors ---
        w_q = persist.tile([128, KO2, 2, HD_LOC], f8, tag="w_q")
        w_k = persist.tile([128, KO2, 2, HD_LOC], f8, tag="w_k")
        w_va = persist.tile([128, KO2, 2, HD_LOC], f8, tag="w_va")
        w_vb = persist.tile([128, KO2, 2, HD_LOC], f8, tag="w_vb")
        w_vc = persist.tile([128, KO2, 2, HD_LOC], f8, tag="w_vc")
        w_o = persist.tile([128, H_LOC, D], f16, tag="w_o")
        kt = persist.tile([128, H_LOC, T], f16, tag="kt")
        vt = persist.tile([128, KO, HD_LOC], f16, tag="vt")
        ctab = persist.tile([128, H_LOC, T], f16, tag="ctab")
        stab = persist.tile([128, H_LOC, T], f16, tag="stab")
        roll = persist.tile([128, 128], f16, tag="roll")
        ones = persist.tile([128, 128], f16, tag="ones")
        tri = persist.tile([128, H_LOC, 128], f16, tag="tri")

        def issue_x(c):
            """Queue the x chunk DMAs for chunk c (weights too on chunk 0)."""
            cs = c * TCH
            x8c = xpool.tile([128, KO2, 2, TCH], f8, tag="x8c", name="x8c")
            xloc = xpool.tile([128, KO2, 2, TCH], f8, tag="xloc", name="xloc")
            if c == 0:
                # ordered so the first PE work (q-proj, rope) unblocks
                # soonest; first transfers split+interleaved so the opening
                # matmul group can start after half the data has landed
                nc.sync.dma_start(x8c[:, :4], x8_d[:, :4, :, cs:cs + TCH])
                nc.sync.dma_start(w_q[:, :4], wq_d[:, :4])
                nc.sync.dma_start(x8c[:, 4:], x8_d[:, 4:, :, cs:cs + TCH])
                nc.sync.dma_start(w_q[:, 4:], wq_d[:, 4:])
                nc.sync.dma_start(w_k[:], wk_d[:])
                nc.sync.dma_start(w_va[:], wva_d[:])
                nc.sync.dma_start(xloc[:], xl_d[:, :, :, cs:cs + TCH])
                nc.sync.dma_start(w_vb[:], wvb_d[:])
                nc.sync.dma_start(w_vc[:], wvc_d[:])
            elif c == 1:
                # constants not needed until attention/c_proj stream AFTER
                # chunk 1's x so its projections unblock sooner
                nc.sync.dma_start(roll[:], roll_d[:])
                nc.sync.dma_start(x8c[:], x8_d[:, :, :, cs:cs + TCH])
                nc.sync.dma_start(ctab[:], ct_d[:])
                nc.sync.dma_start(stab[:], st_d[:])
                nc.sync.dma_start(xloc[:], xl_d[:, :, :, cs:cs + TCH])
                nc.sync.dma_start(tri[:], tri_d[:])
                nc.sync.dma_start(ones[:], ones_d[:])
            elif c == 2:
                nc.sync.dma_start(x8c[:], x8_d[:, :, :, cs:cs + TCH])
                nc.sync.dma_start(xloc[:], xl_d[:, :, :, cs:cs + TCH])
                nc.sync.dma_start(w_o[:], wo_r)
            else:
                nc.sync.dma_start(x8c[:], x8_d[:, :, :, cs:cs + TCH])
                nc.sync.dma_start(xloc[:], xl_d[:, :, :, cs:cs + TCH])
            return (x8c, xloc)

        def rope(sl, cs):
            """RoPE in place: y = x*C + roll64(x)*S', heads fused per DVE op.

            The rolled psum is ACT-copied to fp16 SBUF first so every DVE op
            runs in the all-16-bit 2x mode (a psum-f32 operand would drop the
            multiply to 1x)."""
            rolled = mix_tile()
            for h in range(H_LOC):
                nc.tensor.matmul(rolled[:, h, :], lhsT=roll, rhs=sl[:, h, :],
                                 start=True, stop=True)
            r16 = rtmp.tile([128, H_LOC, TCH], f16, tag="r16", name="r16")
            nc.scalar.copy(out=r16, in_=rolled)
            a = rtmp.tile([128, H_LOC, TCH], f16, tag="ra", name="ra")
            b = rtmp.tile([128, H_LOC, TCH], f16, tag="rb", name="rb")
            nc.vector.tensor_mul(out=a, in0=sl, in1=ctab[:, :, cs:cs + TCH])
            nc.vector.tensor_mul(out=b, in0=r16, in1=stab[:, :, cs:cs + TCH])
            nc.vector.tensor_add(out=sl, in0=a, in1=b)

        def qk_head(w_sb, x8c, dsl, h):
            """One head's q-or-k projection: 16 DoubleRow matmuls + copy."""
            ps = ps_tile()
            for tp in range(2):
                for jko in range(KO2):
                    nc.tensor.matmul(
                        ps[:, tp * 256:(tp + 1) * 256],
                        lhsT=w_sb[:, jko, :, h * 128:(h + 1) * 128],
                        rhs=x8c[:, jko, :, tp * 256:(tp + 1) * 256],
                        start=(jko == 0),
                        stop=(jko == KO2 - 1),
                        perf_mode=DR,
                    )
            nc.scalar.copy(out=dsl, in_=ps)

        def v_half(c, xc, tu):
            """Half a chunk's v projection: 3-pass split-fp8 DoubleRow.

            v = x8*Wva(64w) + xlo(8dx)*Wvb(8w) + x8*Wvc(64dw); PSUM holds
            64*v, the evacuation copy scales by 1/64."""
            x8c, xloc = xc
            passes = ((x8c, w_va), (xloc, w_vb), (x8c, w_vc))
            ps = ps_tile()
            for tt in (2 * tu, 2 * tu + 1):
                sub = ps[:, (tt % 2) * HD_LOC:(tt % 2 + 1) * HD_LOC]
                for pi, (xt, wt) in enumerate(passes):
                    for jko in range(KO2):
                        nc.tensor.matmul(
                            sub,
                            lhsT=xt[:, jko, :, tt * 128:(tt + 1) * 128],
                            rhs=wt[:, jko, :, :],
                            start=(pi == 0 and jko == 0),
                            stop=(pi == 2 and jko == KO2 - 1),
                            perf_mode=DR,
                        )
            gt2 = c * 2 + tu
            nc.scalar.mul(out=vt[:, 2 * gt2:2 * gt2 + 2, :], in_=ps,
                          mul=1.0 / WS)

        def proj_q(c, xc, qtag="qc", do_rope=True):
            """q projection + its rope for t-chunk c."""
            qc = qpool.tile([128, H_LOC, TCH], f16, tag=qtag, name="qc")
            for h in range(H_LOC):
                qk_head(w_q, xc[0], qc[:, h, :], h)
            if do_rope:
                rope(qc[:, :, :], c * TCH)
            return qc

        def kv_quanta(c, xc):
            """k/v projections for chunk c as quanta (PE-heavy, ACT-light) --
            interleaved into the previous chunk's attention span."""
            cs = c * TCH

            def k_head(h):
                qk_head(w_k, xc[0], kt[:, h, cs:cs + TCH], h)

            return [
                lambda: k_head(0),
                lambda: k_head(1),
                lambda: v_half(c, xc, 0),
                lambda: v_half(c, xc, 1),
                lambda: rope(kt[:, :, cs:cs + TCH], cs),
            ]

        def attn_span(q0, W, qc, off, yc, jt_lo=0, jt_hi=None,
                      state=None, ot_pool=None, filler=(), fill_per_jt=1):
            """Causal attention for queries [q0, q0+W), heads interleaved.

            q0 must be 128-aligned; W in {256, 512}. qc holds the chunk's
            roped queries; off is q0's offset within qc/yc."""
            d0 = q0 // 128          # first diagonal j-tile
            n_jt = d0 + W // 128
            if state is None:
                ots = [ps_tile(ot_pool or psum_ot) for _ in range(H_LOC)]
                vecsums = [spool.tile([128, H_LOC, TCH], f16,
                                      tag=f"vecsum{par}", name="vecsum")
                           for par in range(2)]
            else:
                ots, vecsums = state
            if jt_hi is None:
                jt_hi = n_jt
            filler = iter(filler) if not hasattr(filler, "__next__") else filler
            for jt in range(jt_lo, jt_hi):
                # interleave deferred work (previous chunk's c_proj) into the
                # jt loop: PE's stream is in-order per engine, so this is the
                # only way it can fill the exp-gated gaps between j-tiles
                for _ in range(fill_per_jt):
                    q = next(filler, None)
                    if q is not None:
                        q()
                pair = mix_tile()
                m = jt - d0
                # diagonal block: cols < 128m fully masked -- never written,
                # never read (partial-width ops)
                lo = 128 * m if m > 0 else 0
                # score matmul skips dead columns too, but only while the
                # moving dim stays >= 256 (full rate)
                slo = lo if W - lo >= 256 else 0
                for h in range(H_LOC):
                    nc.tensor.matmul(
                        pair[:, h, slo:W],
                        lhsT=kt[:, h, jt * 128:(jt + 1) * 128],
                        rhs=qc[:, h, off + slo:off + W],
                        start=True,
                        stop=True,
                    )
                pt = ptpool.tile([128, H_LOC, TCH], f16, tag="pt", name="pt")
                # both heads in ONE activation call (strided AP when lo > 0);
                # q/k carry the 64x host prescale each -> 1/4096 here
                nc.scalar.activation(out=pt[:, :, lo:W], in_=pair[:, :, lo:W],
                                     func=EXP, scale=SCALE / (WS * WS))
                if m >= 0:
                    # mask the diagonal block, both heads in one op
                    nc.vector.tensor_mul(
                        out=pt[:, :, 128 * m:128 * (m + 1)],
                        in0=pt[:, :, 128 * m:128 * (m + 1)],
                        in1=tri[:],
                    )
                # probability row-sum accumulator (all DVE: f16 runs 2x and
                # GPSIMD's 0.42-efficiency adds would chain on the critical
                # path). jt==0 initializes via copy; on q0=0 spans jt==1 is
                # diagonal with cols < 128 unwritten, so never full-copy there.
                vs = vecsums[0]
                if jt == 0:
                    nc.vector.tensor_copy(out=vs[:, :, :W], in_=pt[:, :, :W])
                else:
                    nc.vector.tensor_add(out=vs[:, :, lo:W], in0=vs[:, :, lo:W],
                                         in1=pt[:, :, lo:W])
                for h in range(H_LOC):
                    # partial-width diagonal writes skip the (bank-granular)
                    # psum group check -- EXCEPT the last j-tile, whose
                    # stop must be bookkept so the ymul read sees a closed
                    # group
                    nc.tensor.matmul(
                        ots[h][:, lo:W],
                        lhsT=vt[:, jt, h * 128:(h + 1) * 128],
                        rhs=pt[:, h, lo:W],
                        start=(jt == 0),
                        stop=(jt == n_jt - 1),
                        skip_group_check=(lo > 0 and jt != n_jt - 1),
                    )
            if jt_hi < n_jt:
                return (ots, vecsums), filler
            # denominator: all-ones matmuls -> column sums on all partitions;
            # one psum tile + one fused reciprocal for both heads
            den = mix_tile()
            for h in range(H_LOC):
                nc.tensor.matmul(den[:, h, :W], lhsT=ones,
                                 rhs=vecsums[0][:, h, :W],
                                 start=True, stop=True)
            # evacuate the PV accumulators to fp16 SBUF on ACT (overlaps the
            # den/recip chain), then the normalize multiplies run in DVE's
            # all-16-bit 2x mode
            oc = rtmp.tile([128, H_LOC, TCH], f16, tag="oc", name="oc")
            for h in range(H_LOC):
                nc.scalar.copy(out=oc[:, h, :W], in_=ots[h][:, :W])
            recipb = rtmp.tile([128, H_LOC, TCH], f16, tag="recipb",
                               name="recipb")
            with nc.allow_low_precision(reason="denominators are O(100) and "
                                        "fp16 keeps 3+ digits"):
                nc.vector.reciprocal(out=recipb[:, :, :W], in_=den[:, :, :W])
            for h in range(H_LOC):
                nc.vector.tensor_mul(out=yc[:, h, off:off + W],
                                     in0=oc[:, h, :W], in1=recipb[:, h, :W])
            return filler

        def cproj_quanta(q0, W, yc, off, pools=None, dve_only=False):
            """Partial c_proj for rows [q0, q0+W) as a list of work quanta.

            Each quantum emits half a 128-row tile (4 matmuls + one [128,
            1024] PSUM evacuation + its output DMA); the caller threads them
            into an attention span's jt loop so PE fills exp-gated gaps.
            pools: optional psum pool rotation (tail c_projs run when the
            proj/attention pools are idle -- deeper pipelining)."""
            obs = {}

            def quantum(tt, half, pool):
                gt = q0 // 128 + tt
                if half == 0:
                    obs[tt] = opool.tile([128, D], f16, tag="ob", name="ob")
                if pool is None:
                    ps = mix_tile()
                else:
                    ps = pool.tile([128, H_LOC, TCH], f32, tag="cp", name="cp")
                for nk in range(2):
                    nck = half * 2 + nk
                    for h in range(H_LOC):
                        nc.tensor.matmul(
                            ps[:, nk, :],
                            lhsT=yc[:, h,
                                    off + tt * 128:off + (tt + 1) * 128],
                            rhs=w_o[:, h, nck * 512:(nck + 1) * 512],
                            start=(h == 0),
                            stop=(h == H_LOC - 1),
                        )
                # evacuate [128, 1024] in one instr; alternate ACT/DVE;
                # one full-row DMA per gt (HWDGE descriptor-gen is a serial
                # 625ns/DMA resource worth conserving)
                osl = obs[tt][:, half * 1024:(half + 1) * 1024]
                if half == 0:
                    nc.scalar.copy(out=osl, in_=ps)
                else:
                    nc.vector.tensor_copy(out=osl, in_=ps)
                    nc.sync.dma_start(
                        out_d[gt * 128:(gt + 1) * 128, :], obs[tt][:])

            def quantum_nck(tt, nck, pool):
                """Pool-rotation variant: one nck per quantum, [128, 512]
                psum tiles from the (tail-idle) proj/attention rings."""
                gt = q0 // 128 + tt
                if nck == 0:
                    obs[tt] = opool.tile([128, D], f16, tag="ob", name="ob")
                ps = ps_tile(pool)
                for h in range(H_LOC):
                    nc.tensor.matmul(
                        ps,
                        lhsT=yc[:, h, off + tt * 128:off + (tt + 1) * 128],
                        rhs=w_o[:, h, nck * 512:(nck + 1) * 512],
                        start=(h == 0),
                        stop=(h == H_LOC - 1),
                    )
                osl = obs[tt][:, nck * 512:(nck + 1) * 512]
                if nck % 2 == 0 and not dve_only:
                    nc.scalar.copy(out=osl, in_=ps)
                else:
                    nc.vector.tensor_copy(out=osl, in_=ps)
                if nck % 2 == 1:
                    # per-half DMAs: the tail has HWDGE to spare and the
                    # earlier transfer start shortens the final drain
                    nc.sync.dma_start(
                        out_d[gt * 128:(gt + 1) * 128,
                              (nck - 1) * 512:(nck + 1) * 512],
                        obs[tt][:, (nck - 1) * 512:(nck + 1) * 512])

            if pools:
                return [
                    (lambda tt=tt, nck=nck,
                     pool=pools[(4 * tt + nck) % len(pools)]:
                     quantum_nck(tt, nck, pool))
                    for tt in range(W // 128) for nck in range(4)
                ]
            return [
                (lambda tt=tt, half=half: quantum(tt, half, None))
                for tt in range(W // 128) for half in range(2)
            ]

        def drain(filler):
            filler = iter(filler) if not hasattr(filler, "__next__") else filler
            for q in filler:
                if q is not None:
                    q()

        # Emission order: projections stream in chunk order; each attention
        # chunk is emitted as soon as its projections exist. The last chunk
        # splits q from k/v so its early j-tiles overlap the projections, and
        # chunk 0's (tiny) attention is saved for the very end so the serial
        # tail after the final projections is as short as possible.
        xc0 = issue_x(0)
        qc0 = proj_q(0, xc0, qtag="qc0", do_rope=False)
        kv0 = kv_quanta(0, xc0)
        drain(kv0[:-1])          # k heads + v halves (the rope quantum is last)
        yc0 = ypool.tile([128, H_LOC, TCH], f16, tag="yc0", name="yc0")
        xc1 = issue_x(1)         # also queues the rope-table DMAs
        kv0[-1]()                # rope(k0): tables are now emitted before it
        rope(qc0[:, :, :], 0)
        qc1 = proj_q(1, xc1)
        drain(kv_quanta(1, xc1))
        yc1 = ypool.tile([128, H_LOC, TCH], f16, tag="yc", name="yc")
        xc2 = issue_x(2)
        # chunk c+1's k/v projections interleave into chunk c's attention:
        # they are PE-dense but ACT/DVE-light, exactly what the exp-gated
        # jt loop can absorb
        attn_span(TCH, TCH, qc1, 0, yc1, filler=kv_quanta(2, xc2))
        qc2 = proj_q(2, xc2)
        drain(cproj_quanta(TCH, TCH, yc1, 0))
        yc2 = ypool.tile([128, H_LOC, TCH], f16, tag="yc", name="yc")
        xc3 = issue_x(3)
        attn_span(2 * TCH, TCH, qc2, 0, yc2, filler=kv_quanta(3, xc3))
        qc3 = proj_q(3, xc3)
        yc3 = ypool.tile([128, H_LOC, TCH], f16, tag="yc", name="yc")
        # chunk 0's (tiny) attention runs in the pre-attn3 window, its PV
        # accumulators in the now-idle proj psum ring; its c_proj then fills
        # attn3's exp-gated jt loop with DVE-only evacuations (ACT is
        # exp-saturated there). It goes BEFORE cproj2's drain so its score
        # pairs aren't stuck behind cproj2's slow mix-ring evacuations.
        attn_span(0, TCH, qc0, 0, yc0, ot_pool=psum_p)
        drain(cproj_quanta(2 * TCH, TCH, yc2, 0))
        q00 = cproj_quanta(0, TCH, yc0, 0, pools=(psum_p,), dve_only=True)
        # pull one evacuation every OTHER j-tile: DVE also carries the
        # vecsum chain there and saturates at 1/jt
        paced = (q for pair in zip(q00, [None] * len(q00)) for q in pair)
        rest = attn_span(3 * TCH, TCH, qc3, 0, yc3, filler=paced)
        drain(rest)
        drain(cproj_quanta(3 * TCH, TCH, yc3, 0, pools=(psum_ot, psum_p)))

    nc.compile()
    _CACHE["nc"] = nc
    return nc


def host_inputs(x, Wq, Wk, Wv, Wo):
    """Per-core input dicts (host-side shard + transpose + quantize + tables)."""
    import ml_dtypes

    f8 = ml_dtypes.float8_e4m3
    bf16 = ml_dtypes.bfloat16

    def pack_x8(a):  # (D, T) f32 -> (128, KO2, 2, T) fp8 DoubleRow layout
        return np.ascontiguousarray(
            a.reshape(KO2, 2, 128, T).transpose(2, 0, 1, 3)).astype(f8)

    x2 = np.ascontiguousarray(x.reshape(T, D).T).astype(np.float32)  # (D, T)
    x8 = pack_x8(x2)
    # fp8 residual (scaled 8x) for the v projection's second pass
    x8lo = pack_x8(
        8.0 * (x2 - x8.transpose(1, 2, 0, 3).reshape(D, T).astype(np.float32)))

    half = DH // 2  # 64
    af = (1.0 / 1024.0) ** np.linspace(0.0, 1.0, DH // 4, dtype=np.float32)
    af = np.concatenate([af, np.zeros(DH // 4, np.float32)])         # (64,)
    theta = np.arange(T, dtype=np.float32)[:, None] * af[None, :]    # (T, 64)
    cos = np.cos(theta).T.astype(np.float32)                         # (64, T)
    sin = np.sin(theta).T.astype(np.float32)
    ctab1 = np.concatenate([cos, cos], axis=0)                       # (128, T)
    stab1 = np.concatenate([sin, -sin], axis=0)
    # duplicated per head for head-fused rope ops: (128, H_LOC, T)
    ctab = np.repeat(ctab1[:, None, :], H_LOC, axis=1).astype(np.float16)
    stab = np.repeat(stab1[:, None, :], H_LOC, axis=1).astype(np.float32)

    roll = np.zeros((128, 128), np.float16)
    for p in range(128):
        roll[p, (p + half) % 128] = 1.0
    ones = np.ones((128, 128), np.float16)
    tri1 = np.triu(np.ones((128, 128), np.float16))  # tri[j, i] = i >= j
    tri = np.repeat(tri1[:, None, :], H_LOC, axis=1)

    shared = {
        "x8": x8, "x8lo": x8lo, "ctab": ctab, "stab": stab,
        "roll": roll, "ones": ones, "tri": tri,
    }

    def pack_pre(wt):  # pre-scaled (D, HD_LOC) f32 -> DoubleRow fp8 layout
        return np.ascontiguousarray(
            wt.reshape(KO2, 2, 128, HD_LOC).transpose(2, 0, 1, 3)).astype(f8)

    def pack_w8(w):  # (HD_LOC, D) slice -> (128, KO2, 2, HD_LOC) fp8, x WS
        return pack_pre((w.T * WS).astype(np.float32))

    in_maps = []
    for c in range(N_CORES):
        sl = slice(c * HD_LOC, (c + 1) * HD_LOC)
        wv_t = Wv[sl, :].T.astype(np.float32)               # (D, HD_LOC)
        wv8a = pack_pre(wv_t * WS)
        # residual of the 64x-quantized Wv, itself scaled 64x
        wv_res = wv_t - wv8a.transpose(1, 2, 0, 3).reshape(D, HD_LOC).astype(
            np.float32) / WS
        in_maps.append({
            **shared,
            "wq8": pack_w8(Wq[sl, :]),
            "wk8": pack_w8(Wk[sl, :]),
            "wv8a": wv8a,
            "wv8b": pack_pre(wv_t * 8.0),
            "wv8c": pack_pre(wv_res * WS),
            "wo16": np.ascontiguousarray((Wo[:, sl] / 3.0).T).astype(np.float16),
        })
    return in_maps


def _get_runner():
    """Build the program + a persistent jitted SPMD executable (once)."""
    if "runner" in _CACHE:
        return _CACHE["runner"]

    import jax
    import concourse.mybir as mybir
    from concourse.bass2jax import (
        _bass_exec_p,
        install_neuronx_cc_hook,
        partition_id_tensor,
    )
    from jax.experimental.shard_map import shard_map
    from jax.sharding import Mesh, PartitionSpec

    nc = build_program()
    install_neuronx_cc_hook()
    assert nc.dbg_addr is None
    pid_name = nc.partition_id_tensor.name if nc.partition_id_tensor else None

    in_names, out_names, out_avals, zero_outs = [], [], [], []
    for alloc in nc.m.functions[0].allocations:
        if not isinstance(alloc, mybir.MemoryLocationSet):
            continue
        name = alloc.memorylocations[0].name
        if alloc.kind == "ExternalInput":
            if name != pid_name:
                in_names.append(name)
        elif alloc.kind == "ExternalOutput":
            out_names.append(name)
            shape = tuple(alloc.tensor_shape)
            dtype = mybir.dt.np(alloc.dtype)
            out_avals.append(jax.core.ShapedArray(shape, dtype))
            zero_outs.append(np.zeros(shape, dtype))
    n_params = len(in_names)
    all_names = list(in_names) + list(out_names)
    if pid_name is not None:
        all_names.append(pid_name)
    donate = tuple(range(n_params, n_params + len(out_names)))

    def _body(*args):
        operands = list(args)
        if pid_name is not None:
            operands.append(partition_id_tensor())
        outs = _bass_exec_p.bind(
            *operands,
            out_avals=tuple(out_avals),
            in_names=tuple(all_names),
            out_names=tuple(out_names),
            lowering_input_output_aliases=(),
            sim_require_finite=True,
            sim_require_nnan=True,
            nc=nc,
        )
        return tuple(outs)

    devices = jax.devices()[:N_CORES]
    mesh = Mesh(np.asarray(devices), ("core",))
    in_specs = (PartitionSpec("core"),) * (n_params + len(out_names))
    out_specs = (PartitionSpec("core"),) * len(out_names)
    fn = jax.jit(
        shard_map(_body, mesh=mesh, in_specs=in_specs, out_specs=out_specs,
                  check_rep=False),
        donate_argnums=donate,
        keep_unused=True,
    )
    runner = (fn, in_names, out_names, out_avals, zero_outs)
    _CACHE["runner"] = runner
    return runner


def run_spmd(in_maps):
    """Execute the SPMD program; returns per-core output dicts."""
    fn, in_names, out_names, out_avals, zero_outs = _get_runner()
    concat_in = [
        np.concatenate([np.asarray(in_maps[c][n]) for c in range(N_CORES)], axis=0)
        for n in in_names
    ]
    concat_zeros = [
        np.zeros((N_CORES * z.shape[0], *z.shape[1:]), z.dtype) for z in zero_outs
    ]
    out_arrs = fn(*concat_in, *concat_zeros)
    return [
        {n: np.asarray(out_arrs[i]).reshape(N_CORES, *out_avals[i].shape)[c]
         for i, n in enumerate(out_names)}
        for c in range(N_CORES)
    ]


def kernel(x, Wq, Wk, Wv, Wo):
    in_maps = host_inputs(np.asarray(x), np.asarray(Wq), np.asarray(Wk),
                          np.asarray(Wv), np.asarray(Wo))
    results = run_spmd(in_maps)
    out = results[0]["outp"].astype(np.float64)
    for c in range(1, N_CORES):
        out += results[c]["outp"].astype(np.float64)
    return out.astype(np.float32).reshape(1, T, D)
